# revision 20
# baseline (speedup 1.0000x reference)
"""Trainium2 Bass kernel for nn_Attention_44994077393310.

Multi-head attention (B=8, N=2048, C=768, H=4, Dh=192) with input projections,
softmax attention, and output projection with bias.

Sharding: pure data-parallel over the batch dim - each of the 8 NeuronCores
computes one batch element end-to-end (weights replicated). No collectives.

v2: all matmul operands are bf16 (inputs cast on the host). Rationale from the
v1 (fp32r) trace: the PE was 93% busy but ~1/3 of its time was exposed
LDWEIGHTS - fp32 weights disable the HW fast-weight-load path and every
512-col matmul serialized a ~110ns weight load. bf16 enables FWL, halves DMA
bytes, and the kernel is restructured so consecutive matmul pairs share one
stationary operand (1024-wide q superchunks processed as two 512-col moving
halves per weight load), halving the LDWEIGHTS count.

Layout strategy (unchanged from v1): q/k/v and weights are pre-transposed on
the host so every DMA lands operands with the contraction dim on partitions.
Scores are computed transposed S_T[key, q] with exp on ScalarE (scale folded
in); a ones column in vh makes softmax denominators fall out of the U = es@v
matmuls; U is evacuated RAW to SBUF (f32) as soon as a head finishes so the
single-buffered U psum frees immediately, and the slow 1-partition RECIPROCAL
+ broadcast + normalize runs on DVE hidden under the next head / the output
projection.

PSUM plan (8 banks): one pool - tag "big" [128,1024]f32 x2 bufs (4 banks,
used by scores/projections/bc broadcast), tags "ua" [128,1024] + "ub"
[65,1024] x1 buf (4 banks, the per-head U accumulators).
"""

import numpy as np

B = 8
N = 2048
C = 768
H = 4
DH = 192
SCALE = DH ** -0.5

NSC = 2                 # superchunks of 1024 over the sequence
SC = N // NSC           # 1024
HF = SC // 2            # 512 (moving-operand half width)
CC = C // 128           # 6 channel chunks
KT = N // 128           # 16 k-tiles
NWARM = 44

_BUILT = None


def _dest_of(cp):
    h, dd = divmod(cp, DH)
    if dd < 128:
        return ("a", h, dd)
    return ("b", h // 2, (h % 2) * 64 + (dd - 128))


def _jc_segments(jc):
    """Merged PSUM->head-major copy segments for projection j-chunk jc."""
    segs = []
    for p0 in range(0, 128, 64):
        kind, idx, dlo = _dest_of(128 * jc + p0)
        if segs and segs[-1][2] == kind and segs[-1][3] == idx and \
                segs[-1][4] + (segs[-1][1] - segs[-1][0]) == dlo:
            segs[-1] = (segs[-1][0], p0 + 64, kind, idx, segs[-1][4])
        else:
            segs.append((p0, p0 + 64, kind, idx, dlo))
    return segs


def _build():
    from contextlib import ExitStack

    import concourse.mybir as mybir
    import concourse.tile as tile
    from concourse import bacc

    F32 = mybir.dt.float32
    F32R = mybir.dt.float32r
    BF16 = mybir.dt.bfloat16
    AF = mybir.ActivationFunctionType

    nc = bacc.Bacc("TRN2", target_bir_lowering=False, debug=False)
    qt_d = nc.dram_tensor("qT", [C, N], BF16, kind="ExternalInput").ap()
    kt_d = nc.dram_tensor("kT", [C, N], BF16, kind="ExternalInput").ap()
    vt_d = nc.dram_tensor("vT", [C, N], BF16, kind="ExternalInput").ap()
    wqt_d = nc.dram_tensor("WqT", [C, C], BF16, kind="ExternalInput").ap()
    wkt_d = nc.dram_tensor("WkT", [C, C], BF16, kind="ExternalInput").ap()
    wvt_d = nc.dram_tensor("WvT", [C, C], BF16, kind="ExternalInput").ap()
    wpt_d = nc.dram_tensor("WpT", [C, C], BF16, kind="ExternalInput").ap()
    bp_d = nc.dram_tensor("bp", [C], F32, kind="ExternalInput").ap()
    y_d = nc.dram_tensor("y", [N, C], F32, kind="ExternalOutput").ap()

    with tile.TileContext(nc) as tc, ExitStack() as ctx:
        const = ctx.enter_context(tc.tile_pool(name="const", bufs=1))
        wqp = ctx.enter_context(tc.tile_pool(name="wqp", bufs=1))
        khp = ctx.enter_context(tc.tile_pool(name="khp", bufs=1))
        vhp = ctx.enter_context(tc.tile_pool(name="vhp", bufs=1))
        xtp = ctx.enter_context(tc.tile_pool(name="xtp", bufs=3))
        ps = ctx.enter_context(tc.tile_pool(name="ps", bufs=2, space="PSUM"))

        def big_tile(nm):
            return ps.tile([128, SC], F32, tag="big", name=nm, bufs=2)

        ones_col = const.tile([128, H], BF16, tag="ones_col", name="ones_col")
        nc.vector.memset(ones_col[:], 1.0)
        ones_row_f = const.tile([1, 128], F32, tag="ones_row_f",
                                name="ones_row_f")
        nc.vector.memset(ones_row_f[:], 1.0)
        ones_row = const.tile([1, 128], F32R, tag="ones_row", name="ones_row")
        nc.vector.tensor_copy(ones_row[:], ones_row_f[:])

        # PE warm-up: dependency-free matmuls so the HAM clock gate opens
        # while the first DMAs stream in.
        warm_w_f = const.tile([128, 128], F32, tag="warm_w_f", name="warm_w_f")
        nc.vector.memset(warm_w_f[:], 0.5)
        warm_w = const.tile([128, 128], BF16, tag="warm_w", name="warm_w")
        nc.vector.tensor_copy(warm_w[:], warm_w_f[:])
        warm_x = const.tile([128, HF], BF16, tag="warm_x", name="warm_x")
        for i in range(4):
            nc.vector.tensor_copy(warm_x[:, i * 128:(i + 1) * 128], warm_w_f[:])
        for r in range(NWARM):
            wp = ps.tile([128, SC], F32, tag="ua", name="warm_ps", bufs=1)
            nc.tensor.matmul(wp[:, 0:HF], warm_w[:], warm_x[:],
                             start=True, stop=True)

        # ---- persistent weights (direct loads, no transposes) -------------
        WqT = wqp.tile([128, CC, C], BF16, tag="wqt", name="wqt")
        WpT_a = wqp.tile([128, H, C], BF16, tag="wpa", name="wpa")
        WpT_b = [wqp.tile([128, C], BF16, tag=f"wpb{g}", name=f"wpb{g}")
                 for g in range(2)]
        bias_bc = wqp.tile([128, C], F32, tag="bias_bc", name="bias_bc")

        khT_a = [khp.tile([128, N], BF16, tag=f"kha{h}", name=f"kha{h}")
                 for h in range(H)]
        khT_b = [khp.tile([128, N], BF16, tag=f"khb{g}", name=f"khb{g}")
                 for g in range(2)]
        vh = [vhp.tile([128, H, DH + 1], BF16, tag=f"vh{nt}", name=f"vh{nt}")
              for nt in range(KT)]

        def load_wT_grouped(dest, w_dram):
            # dest[p, cc, j] = W.T[cc*128+p, j]
            nc.gpsimd.dma_start(
                dest[:],
                w_dram.rearrange("(cc p) j -> p cc j", p=128))

        def seg_dest(kind, idx, dlo, dhi, a_tiles, b_tiles, col_lo, col_hi):
            t = a_tiles[idx] if kind == "a" else b_tiles[idx]
            return t[dlo:dhi, col_lo:col_hi]

        def in_proj(w_tiles, xTt, a_tiles, b_tiles, n0):
            # out.T[j, n0:n0+SC] head-major packed; one weight load feeds the
            # two 512-col moving halves.
            for jc in range(CC):
                p = big_tile("pj")
                for cc in range(CC):
                    w = w_tiles[:, cc, jc * 128:(jc + 1) * 128]
                    nc.tensor.matmul(p[:, 0:HF], w, xTt[:, cc, 0:HF],
                                     start=(cc == 0), stop=(cc == CC - 1))
                    nc.tensor.matmul(p[:, HF:SC], w, xTt[:, cc, HF:SC],
                                     start=(cc == 0), stop=(cc == CC - 1))
                for (plo, phi, kind, idx, dlo) in _jc_segments(jc):
                    nc.vector.tensor_copy(
                        seg_dest(kind, idx, dlo, dlo + (phi - plo),
                                 a_tiles, b_tiles, n0, n0 + SC),
                        p[plo:phi, :])

        # ---- phase 1: stage k, v ------------------------------------------
        with tc.tile_pool(name="wkv", bufs=1) as wkv:
            WkT = wkv.tile([128, CC, C], BF16, tag="wkt", name="wkt")
            WvT = wkv.tile([128, CC, C], BF16, tag="wvt", name="wvt")
            load_wT_grouped(WkT, wkt_d)

            def load_wq():
                load_wT_grouped(WqT, wqt_d)

            def load_wp_bias():
                # wpt_d is host-packed head-major: rows 0..511 = per-head
                # dd 0..127 (h-major), rows 512..639 / 640..767 = the packed
                # b-tiles (dd 128..191 of heads 0,1 / 2,3).
                nc.gpsimd.dma_start(
                    WpT_a[:],
                    wpt_d[0:512, :].rearrange("(h p) j -> p h j", p=128))
                for g in range(2):
                    nc.gpsimd.dma_start(
                        WpT_b[g][:], wpt_d[512 + g * 128:512 + (g + 1) * 128, :])
                bp_row = wkv.tile([1, C], F32, tag="bp_row", name="bp_row")
                bp_row_r = wkv.tile([1, C], F32R, tag="bp_row_r",
                                    name="bp_row_r")
                nc.sync.dma_start(bp_row[:], bp_d[None, :])
                nc.vector.tensor_copy(bp_row_r[:], bp_row[:])
                pb = big_tile("pb")
                nc.tensor.matmul(pb[:, 0:HF], ones_row[:], bp_row_r[:, 0:HF],
                                 start=True, stop=True)
                nc.tensor.matmul(pb[:, HF:C], ones_row[:], bp_row_r[:, HF:C],
                                 start=True, stop=True)
                nc.scalar.copy(bias_bc[:], pb[:, 0:C])

            for sc in range(NSC):
                n0 = sc * SC
                kTt = xtp.tile([128, CC, SC], BF16, tag="xT", name="kTt")
                nc.gpsimd.dma_start(
                    kTt[:],
                    kt_d[:, n0:n0 + SC].rearrange("(cc p) n -> p cc n", p=128))
                if sc == 0:
                    # WvT queued after the first k staging chunk so the first
                    # k-projection matmuls start ~5us earlier.
                    load_wT_grouped(WvT, wvt_d)
                in_proj(WkT, kTt, khT_a, khT_b, n0)
                vTt = xtp.tile([128, CC, SC], BF16, tag="xT", name="vTt")
                nc.gpsimd.dma_start(
                    vTt[:],
                    vt_d[:, n0:n0 + SC].rearrange("(cc p) n -> p cc n", p=128))
                for ntl in range(SC // 128):
                    nt = sc * (SC // 128) + ntl
                    p = big_tile("pv")
                    for cc in range(CC):
                        xw = vTt[:, cc, ntl * 128:(ntl + 1) * 128]
                        nc.tensor.matmul(p[:, 0:HF], xw, WvT[:, cc, 0:HF],
                                         start=(cc == 0), stop=(cc == CC - 1))
                        nc.tensor.matmul(p[:, HF:C], xw, WvT[:, cc, HF:C],
                                         start=(cc == 0), stop=(cc == CC - 1))
                    nc.vector.tensor_copy(
                        vh[nt][:, :, 0:DH],
                        p[:, 0:C].rearrange("p (h d) -> p h d", h=H))
                    nc.vector.tensor_copy(
                        vh[nt][:, :, DH:DH + 1],
                        ones_col[:].rearrange("p (h o) -> p h o", h=H))
                if sc == 0:
                    load_wq()
                    load_wp_bias()

        # ---- phase 2: per-superchunk attention + output projection --------
        qhp = ctx.enter_context(tc.tile_pool(name="qhp", bufs=1))
        esp = ctx.enter_context(tc.tile_pool(name="esp", bufs=3))
        xop = ctx.enter_context(tc.tile_pool(name="xop", bufs=1))
        scp = ctx.enter_context(tc.tile_pool(name="scp", bufs=2))
        yp = ctx.enter_context(tc.tile_pool(name="yp", bufs=2))

        qhT_a = [qhp.tile([128, SC], BF16, tag=f"qha{h}", name=f"qha{h}")
                 for h in range(H)]
        qhT_b = [qhp.tile([128, SC], BF16, tag=f"qhb{g}", name=f"qhb{g}")
                 for g in range(2)]
        # raw (unnormalized) U, f32 to keep a single bf16 rounding on x
        ur_a = [xop.tile([128, SC], F32, tag=f"ura{h}", name=f"ura{h}")
                for h in range(H)]
        ur_b = [xop.tile([128, SC], F32, tag=f"urb{g}", name=f"urb{g}")
                for g in range(2)]
        rs_t = xop.tile([128, SC], F32, tag="rs", name="rs")
        rs = [rs_t[32 * h:32 * h + 1, :] for h in range(H)]
        xT_a = [xop.tile([128, SC], BF16, tag=f"xta{h}", name=f"xta{h}")
                for h in range(H)]
        xT_b = [xop.tile([128, SC], BF16, tag=f"xtb{g}", name=f"xtb{g}")
                for g in range(2)]

        def q_load(sc):
            n0 = sc * SC
            qTt = xtp.tile([128, CC, SC], BF16, tag="xT", name="qTt")
            nc.gpsimd.dma_start(
                qTt[:],
                qt_d[:, n0:n0 + SC].rearrange("(cc p) n -> p cc n", p=128))
            return qTt

        def finalize_recip(fh, src=None):
            # 1-partition RECIPROCAL of the rowsum row on DVE (iterative
            # divide, ~6.7us at 1024 wide) - slow but fully hidden under the
            # next head's score loop / the output projection.
            recip = scp.tile([1, SC], F32R, tag="recip", name="recip")
            with nc.allow_low_precision(reason="softmax denom recip f32r"):
                nc.vector.reciprocal(recip[:],
                                     rs[fh][:] if src is None else src)
            return recip

        def finalize_muls(fh, recip):
            # broadcast 1/rowsum across partitions with a rank-1 ones matmul,
            # then normalize the raw U tiles into the bf16 stationaries for
            # the output projection.
            g, blo = fh // 2, (fh % 2) * 64
            bc_ps = big_tile("bcp")
            nc.tensor.matmul(bc_ps[:, 0:HF], ones_row[:], recip[:, 0:HF],
                             start=True, stop=True)
            nc.tensor.matmul(bc_ps[:, HF:SC], ones_row[:], recip[:, HF:SC],
                             start=True, stop=True)
            bc = scp.tile([128, SC], F32, tag="bc", name="bc")
            nc.scalar.copy(bc[:], bc_ps[:])
            nc.vector.tensor_mul(xT_a[fh][:], ur_a[fh][:], bc[:])
            nc.vector.tensor_mul(xT_b[g][blo:blo + 64, :],
                                 ur_b[g][blo:blo + 64, :], bc[blo:blo + 64, :])

        def attention(sc):
            # (h, kt) flattened with the score stream leading the av stream
            # by two groups, so the next head's first exps are already in
            # flight when its avs begin - no per-head-boundary PE gap.
            def scores(h, kt):
                g, blo = h // 2, (h % 2) * 64
                s = big_tile("s")
                wa = khT_a[h][:, kt * 128:(kt + 1) * 128]
                qa = qhT_a[h]
                nc.tensor.matmul(s[:, 0:HF], wa, qa[:, 0:HF],
                                 start=True, stop=False)
                nc.tensor.matmul(s[:, HF:SC], wa, qa[:, HF:SC],
                                 start=True, stop=False)
                wb = khT_b[g][blo:blo + 64, kt * 128:(kt + 1) * 128]
                qb = qhT_b[g]
                nc.tensor.matmul(s[:, 0:HF], wb, qb[blo:blo + 64, 0:HF],
                                 start=False, stop=True)
                nc.tensor.matmul(s[:, HF:SC], wb, qb[blo:blo + 64, HF:SC],
                                 start=False, stop=True)
                es = esp.tile([128, SC], BF16, tag="es", name="es")
                nc.scalar.activation(es[:], s[:], AF.Exp, scale=SCALE)
                return es

            def av(h, kt, u_a, u_b, es):
                va = vh[kt][:, h, 0:128]
                st, sp = (kt == 0), (kt == KT - 1)
                nc.tensor.matmul(u_a[:, 0:HF], va, es[:, 0:HF],
                                 start=st, stop=sp)
                nc.tensor.matmul(u_a[:, HF:SC], va, es[:, HF:SC],
                                 start=st, stop=sp)
                vb = vh[kt][:, h, 128:DH + 1]
                nc.tensor.matmul(u_b[:, 0:HF], vb, es[:, 0:HF],
                                 start=st, stop=sp)
                nc.tensor.matmul(u_b[:, HF:SC], vb, es[:, HF:SC],
                                 start=st, stop=sp)

            seq = [(h, kt) for h in range(H) for kt in range(KT)]
            es_q = [scores(*seq[0]), scores(*seq[1])]
            pend = None
            u = None
            for j, (h, kt) in enumerate(seq):
                if j + 2 < len(seq):
                    es_q.append(scores(*seq[j + 2]))
                if kt == 0:
                    u = (ps.tile([128, SC], F32, tag="ua", name="ua", bufs=1),
                         ps.tile([65, SC], F32, tag="ub", name="ub", bufs=1))
                av(h, kt, *u, es_q.pop(0))
                if kt == 2 and pend is not None:
                    pend = (pend[0], finalize_recip(pend[0]))
                elif kt == 8 and pend is not None:
                    finalize_muls(*pend)
                    pend = None
                elif kt == KT - 1:
                    # evacuate raw U immediately so the single-buffered U
                    # psum frees before the next head's first av matmul.
                    # The rowsum row stays in psum for the LAST head: its
                    # reciprocal runs during the projection phase, when no
                    # next av needs the ub buffer, and skipping the staging
                    # copy shortens the normalization chain.
                    g, blo = h // 2, (h % 2) * 64
                    nc.scalar.copy(ur_a[h][:], u[0][:])
                    nc.vector.tensor_copy(ur_b[g][blo:blo + 64, :],
                                          u[1][0:64, :])
                    if h != H - 1:
                        nc.vector.tensor_copy(rs[h][:], u[1][64:65, :])
                    pend = (h,)
            return pend[0], u[1]

        def q_proj_groups(qTt, scalar_evac=True):
            # one projection group per output j-chunk; emitted lazily so the
            # caller can interleave them with the output projection. PSUM
            # evacuation on ScalarE when these groups run while the pending
            # head's 6.5us RECIPROCAL occupies the DVE FIFO (a DVE evac
            # there would stall the PE's psum-buffer rotation behind it);
            # on DVE for the pre-loop instance, where ScalarE copies would
            # instead delay the first attention exps.
            def group(jc):
                p = big_tile("pq")
                for cc in range(CC):
                    w = WqT[:, cc, jc * 128:(jc + 1) * 128]
                    nc.tensor.matmul(p[:, 0:HF], w, qTt[:, cc, 0:HF],
                                     start=(cc == 0), stop=(cc == CC - 1))
                    nc.tensor.matmul(p[:, HF:SC], w, qTt[:, cc, HF:SC],
                                     start=(cc == 0), stop=(cc == CC - 1))
                cp = nc.scalar.copy if scalar_evac else nc.vector.tensor_copy
                for (plo, phi, kind, idx, dlo) in _jc_segments(jc):
                    cp(seg_dest(kind, idx, dlo, dlo + (phi - plo),
                                qhT_a, qhT_b, 0, SC),
                       p[plo:phi, :])
            return [lambda jc=jc: group(jc) for jc in range(CC)]

        def final_part(p, ntl, heads, start, stop):
            for h in heads:
                g, blo = h // 2, (h % 2) * 64
                xa = xT_a[h][:, ntl * 128:(ntl + 1) * 128]
                nc.tensor.matmul(p[:, 0:HF], xa, WpT_a[:, h, 0:HF],
                                 start=start and h == heads[0], stop=False)
                nc.tensor.matmul(p[:, HF:C], xa, WpT_a[:, h, HF:C],
                                 start=start and h == heads[0], stop=False)
                xb = xT_b[g][blo:blo + 64, ntl * 128:(ntl + 1) * 128]
                nc.tensor.matmul(p[:, 0:HF], xb, WpT_b[g][blo:blo + 64, 0:HF],
                                 start=False, stop=stop and h == heads[-1])
                nc.tensor.matmul(p[:, HF:C], xb, WpT_b[g][blo:blo + 64, HF:C],
                                 start=False, stop=stop and h == heads[-1])

        def final_evac(p, sc, ntl):
            n0 = sc * SC
            ysb = yp.tile([128, C], F32, tag="y", name="y")
            nc.vector.tensor_add(ysb[:], p[:, 0:C], bias_bc[:])
            nc.sync.dma_start(
                y_d[n0 + ntl * 128:n0 + (ntl + 1) * 128, :], ysb[:])

        def final_ntl(sc, ntl):
            p = big_tile("py")
            final_part(p, ntl, list(range(H)), True, True)
            final_evac(p, sc, ntl)

        def finalize_tail(fh, ub_last):
            # no cover work exists after the last superchunk's attention, so
            # split the pending head's normalization into column quarters -
            # final_ntl(ntl) only needs xT columns [ntl*128,(ntl+1)*128), so
            # the output projection starts after a quarter of the reciprocal,
            # read straight from the rowsum row still sitting in psum.
            # The broadcast psum rides the now-idle "ua" bank pair: the "big"
            # bufs hold the two open part_a groups (deadlock otherwise).
            g, blo = fh // 2, (fh % 2) * 64
            recip = scp.tile([1, SC], F32R, tag="recip", name="recip")
            bc = scp.tile([128, SC], F32R, tag="bcr", name="bcr", bufs=1)
            for pi in range(4):
                lo, hi = pi * (SC // 4), (pi + 1) * (SC // 4)
                with nc.allow_low_precision(reason="softmax denom recip"):
                    nc.vector.reciprocal(recip[:, lo:hi],
                                         ub_last[64:65, lo:hi])
                    # partition broadcast on the otherwise-idle GpSimd DMA
                    # engine: no psum bank, no PE matmul, no ScalarE copy in
                    # the chain, so the projection cover never stalls.
                    nc.gpsimd.partition_broadcast(bc[:, lo:hi],
                                                  recip[:, lo:hi])
                    nc.vector.tensor_mul(xT_a[fh][:, lo:hi],
                                         ur_a[fh][:, lo:hi], bc[:, lo:hi])
                    nc.vector.tensor_mul(xT_b[g][blo:blo + 64, lo:hi],
                                         ur_b[g][blo:blo + 64, lo:hi],
                                         bc[blo:blo + 64, lo:hi])

        # q-superchunk pipeline: attention(sc) leaves the last head's
        # normalization pending; the next superchunk's q-projection groups
        # are PE work independent of it and cover the reciprocal chain, then
        # the output projection consumes the normalized stationaries.
        qTt = q_load(0)
        for fn in q_proj_groups(qTt, scalar_evac=False):
            fn()
        for sc in range(NSC):
            if sc + 1 < NSC:
                qt_next = q_load(sc + 1)
            fh, ub_last = attention(sc)
            if sc + 1 < NSC:
                recip = finalize_recip(fh, src=ub_last[64:65, :])
                qp = q_proj_groups(qt_next)
                for fn in qp[0:4]:
                    fn()
                finalize_muls(fh, recip)
                rest = qp[4:]
                for ntl in range(SC // 128):
                    final_ntl(sc, ntl)
                    if rest:
                        rest.pop(0)()
            else:
                # tail: open the first two output-projection groups with the
                # already-normalized heads as cover work, weave the pending
                # head's quartered normalization in, then close them.
                heads_a = [h for h in range(H) if h != fh]
                p0 = big_tile("py")
                final_part(p0, 0, heads_a, True, False)
                p1 = big_tile("py")
                final_part(p1, 1, heads_a, True, False)
                finalize_tail(fh, ub_last)
                final_part(p0, 0, [fh], False, True)
                final_evac(p0, sc, 0)
                final_part(p1, 1, [fh], False, True)
                final_evac(p1, sc, 1)
                for ntl in range(2, SC // 128):
                    final_ntl(sc, ntl)

    nc.compile()
    return nc


def _get_built():
    global _BUILT
    if _BUILT is None:
        _BUILT = _build()
    return _BUILT


def run(inputs, trace=False, **kw):
    """Run on all 8 cores; returns (y [B,N,C] float32, BassKernelResults)."""
    import ml_dtypes
    from concourse.bass_utils import run_bass_kernel_spmd

    nc = _get_built()
    bf16 = ml_dtypes.bfloat16
    f32 = np.float32
    wpt = np.asarray(inputs["Wp"], f32).T  # [c', j]
    wpt_packed = np.concatenate(
        [wpt[h * DH:h * DH + 128] for h in range(H)]
        + [wpt[h * DH + 128:(h + 1) * DH] for h in range(H)])
    shared = {
        "WqT": np.ascontiguousarray(np.asarray(inputs["Wq"], f32).T).astype(bf16),
        "WkT": np.ascontiguousarray(np.asarray(inputs["Wk"], f32).T).astype(bf16),
        "WvT": np.ascontiguousarray(np.asarray(inputs["Wv"], f32).T).astype(bf16),
        "WpT": np.ascontiguousarray(wpt_packed).astype(bf16),
        "bp": np.ascontiguousarray(np.asarray(inputs["bp"], f32)),
    }
    q = np.asarray(inputs["q"], f32)
    k = np.asarray(inputs["k"], f32)
    v = np.asarray(inputs["v"], f32)
    in_maps = []
    for b in range(B):
        m = dict(shared)
        m["qT"] = np.ascontiguousarray(q[b].T).astype(bf16)
        m["kT"] = np.ascontiguousarray(k[b].T).astype(bf16)
        m["vT"] = np.ascontiguousarray(v[b].T).astype(bf16)
        in_maps.append(m)
    res = run_bass_kernel_spmd(nc, in_maps, list(range(B)), trace=trace, **kw)
    y = np.stack([res.results[b]["y"] for b in range(B)]).astype(np.float32)
    return y, res


def kernel(q, k, v, Wq, Wk, Wv, Wp, bp):
    y, _ = run({"q": q, "k": k, "v": v, "Wq": Wq, "Wk": Wk, "Wv": Wv,
                "Wp": Wp, "bp": bp})
    return y


# revision 21
# speedup vs baseline: 1.2006x; 1.2006x over previous
"""Trainium2 Bass kernel for nn_Attention_44994077393310.

Multi-head attention (B=8, N=2048, C=768, H=4, Dh=192) with input projections,
softmax attention, and output projection with bias.

Sharding: pure data-parallel over the batch dim - each of the 8 NeuronCores
computes one batch element end-to-end (weights replicated). No collectives.

v2: all matmul operands are bf16 (inputs cast on the host). Rationale from the
v1 (fp32r) trace: the PE was 93% busy but ~1/3 of its time was exposed
LDWEIGHTS - fp32 weights disable the HW fast-weight-load path and every
512-col matmul serialized a ~110ns weight load. bf16 enables FWL, halves DMA
bytes, and the kernel is restructured so consecutive matmul pairs share one
stationary operand (1024-wide q superchunks processed as two 512-col moving
halves per weight load), halving the LDWEIGHTS count.

Layout strategy (unchanged from v1): q/k/v and weights are pre-transposed on
the host so every DMA lands operands with the contraction dim on partitions.
Scores are computed transposed S_T[key, q] with exp on ScalarE (scale folded
in); a ones column in vh makes softmax denominators fall out of the U = es@v
matmuls; U is evacuated RAW to SBUF (f32) as soon as a head finishes so the
single-buffered U psum frees immediately, and the slow 1-partition RECIPROCAL
+ broadcast + normalize runs on DVE hidden under the next head / the output
projection.

PSUM plan (8 banks): one pool - tag "big" [128,1024]f32 x2 bufs (4 banks,
used by scores/projections/bc broadcast), tags "ua" [128,1024] + "ub"
[65,1024] x1 buf (4 banks, the per-head U accumulators).
"""

import numpy as np

B = 8
N = 2048
C = 768
H = 4
DH = 192
SCALE = DH ** -0.5

NSC = 2                 # superchunks of 1024 over the sequence
SC = N // NSC           # 1024
HF = SC // 2            # 512 (moving-operand half width)
CC = C // 128           # 6 channel chunks
KT = N // 128           # 16 k-tiles
NWARM = 44

_BUILT = None


def _dest_of(cp):
    h, dd = divmod(cp, DH)
    if dd < 128:
        return ("a", h, dd)
    return ("b", h // 2, (h % 2) * 64 + (dd - 128))


def _jc_segments(jc):
    """Merged PSUM->head-major copy segments for projection j-chunk jc."""
    segs = []
    for p0 in range(0, 128, 64):
        kind, idx, dlo = _dest_of(128 * jc + p0)
        if segs and segs[-1][2] == kind and segs[-1][3] == idx and \
                segs[-1][4] + (segs[-1][1] - segs[-1][0]) == dlo:
            segs[-1] = (segs[-1][0], p0 + 64, kind, idx, segs[-1][4])
        else:
            segs.append((p0, p0 + 64, kind, idx, dlo))
    return segs


def _build():
    from contextlib import ExitStack

    import concourse.mybir as mybir
    import concourse.tile as tile
    from concourse import bacc

    F32 = mybir.dt.float32
    F32R = mybir.dt.float32r
    BF16 = mybir.dt.bfloat16
    AF = mybir.ActivationFunctionType

    nc = bacc.Bacc("TRN2", target_bir_lowering=False, debug=False)
    qt_d = nc.dram_tensor("qT", [C, N], BF16, kind="ExternalInput").ap()
    kt_d = nc.dram_tensor("kT", [C, N], BF16, kind="ExternalInput").ap()
    vt_d = nc.dram_tensor("vT", [C, N], BF16, kind="ExternalInput").ap()
    wqt_d = nc.dram_tensor("WqT", [C, C], BF16, kind="ExternalInput").ap()
    wkt_d = nc.dram_tensor("WkT", [C, C], BF16, kind="ExternalInput").ap()
    wvt_d = nc.dram_tensor("WvT", [C, C], BF16, kind="ExternalInput").ap()
    wpt_d = nc.dram_tensor("WpT", [C, C], BF16, kind="ExternalInput").ap()
    bp_d = nc.dram_tensor("bp", [C], F32, kind="ExternalInput").ap()
    y_d = nc.dram_tensor("y", [N, C], F32, kind="ExternalOutput").ap()

    with tile.TileContext(nc) as tc, ExitStack() as ctx:
        const = ctx.enter_context(tc.tile_pool(name="const", bufs=1))
        wqp = ctx.enter_context(tc.tile_pool(name="wqp", bufs=1))
        khp = ctx.enter_context(tc.tile_pool(name="khp", bufs=1))
        vhp = ctx.enter_context(tc.tile_pool(name="vhp", bufs=1))
        xtp = ctx.enter_context(tc.tile_pool(name="xtp", bufs=2))
        ps = ctx.enter_context(tc.tile_pool(name="ps", bufs=2, space="PSUM"))

        def big_tile(nm):
            return ps.tile([128, SC], F32, tag="big", name=nm, bufs=2)

        ones_col = const.tile([128, H], BF16, tag="ones_col", name="ones_col")
        nc.vector.memset(ones_col[:], 1.0)
        ones_row_f = const.tile([1, 128], F32, tag="ones_row_f",
                                name="ones_row_f")
        nc.vector.memset(ones_row_f[:], 1.0)
        ones_row = const.tile([1, 128], F32R, tag="ones_row", name="ones_row")
        nc.vector.tensor_copy(ones_row[:], ones_row_f[:])

        # PE warm-up: dependency-free matmuls so the HAM clock gate opens
        # while the first DMAs stream in.
        warm_w_f = const.tile([128, 128], F32, tag="warm_w_f", name="warm_w_f")
        nc.vector.memset(warm_w_f[:], 0.5)
        warm_w = const.tile([128, 128], BF16, tag="warm_w", name="warm_w")
        nc.vector.tensor_copy(warm_w[:], warm_w_f[:])
        warm_x = const.tile([128, HF], BF16, tag="warm_x", name="warm_x")
        for i in range(4):
            nc.vector.tensor_copy(warm_x[:, i * 128:(i + 1) * 128], warm_w_f[:])
        for r in range(NWARM):
            wp = ps.tile([128, SC], F32, tag="ua", name="warm_ps", bufs=1)
            nc.tensor.matmul(wp[:, 0:HF], warm_w[:], warm_x[:],
                             start=True, stop=True)

        # ---- persistent weights (direct loads, no transposes) -------------
        WqT = wqp.tile([128, CC, C], BF16, tag="wqt", name="wqt")
        WpT_a = wqp.tile([128, H, C], BF16, tag="wpa", name="wpa")
        WpT_b = [wqp.tile([128, C], BF16, tag=f"wpb{g}", name=f"wpb{g}")
                 for g in range(2)]
        bias_bc = wqp.tile([128, C], F32, tag="bias_bc", name="bias_bc")

        khT_a = [khp.tile([128, N], BF16, tag=f"kha{h}", name=f"kha{h}")
                 for h in range(H)]
        khT_b = [khp.tile([128, N], BF16, tag=f"khb{g}", name=f"khb{g}")
                 for g in range(2)]
        vh = [vhp.tile([128, H, DH + 1], BF16, tag=f"vh{nt}", name=f"vh{nt}")
              for nt in range(KT)]

        def load_wT_grouped(dest, w_dram):
            # dest[p, cc, j] = W.T[cc*128+p, j]
            nc.gpsimd.dma_start(
                dest[:],
                w_dram.rearrange("(cc p) j -> p cc j", p=128))

        def seg_dest(kind, idx, dlo, dhi, a_tiles, b_tiles, col_lo, col_hi):
            t = a_tiles[idx] if kind == "a" else b_tiles[idx]
            return t[dlo:dhi, col_lo:col_hi]

        def in_proj(w_tiles, xTt, a_tiles, b_tiles, n0):
            # out.T[j, n0:n0+SC] head-major packed; one weight load feeds the
            # two 512-col moving halves.
            for jc in range(CC):
                p = big_tile("pj")
                for cc in range(CC):
                    w = w_tiles[:, cc, jc * 128:(jc + 1) * 128]
                    nc.tensor.matmul(p[:, 0:HF], w, xTt[:, cc, 0:HF],
                                     start=(cc == 0), stop=(cc == CC - 1))
                    nc.tensor.matmul(p[:, HF:SC], w, xTt[:, cc, HF:SC],
                                     start=(cc == 0), stop=(cc == CC - 1))
                for (plo, phi, kind, idx, dlo) in _jc_segments(jc):
                    nc.vector.tensor_copy(
                        seg_dest(kind, idx, dlo, dlo + (phi - plo),
                                 a_tiles, b_tiles, n0, n0 + SC),
                        p[plo:phi, :])

        # ---- phase 1: stage k, v ------------------------------------------
        with tc.tile_pool(name="wkv", bufs=1) as wkv:
            WkT = wkv.tile([128, CC, C], BF16, tag="wkt", name="wkt")
            WvT = wkv.tile([128, CC, C], BF16, tag="wvt", name="wvt")
            load_wT_grouped(WkT, wkt_d)

            def load_wq():
                load_wT_grouped(WqT, wqt_d)

            def load_wp_bias():
                # wpt_d is host-packed head-major: rows 0..511 = per-head
                # dd 0..127 (h-major), rows 512..639 / 640..767 = the packed
                # b-tiles (dd 128..191 of heads 0,1 / 2,3).
                nc.gpsimd.dma_start(
                    WpT_a[:],
                    wpt_d[0:512, :].rearrange("(h p) j -> p h j", p=128))
                for g in range(2):
                    nc.gpsimd.dma_start(
                        WpT_b[g][:], wpt_d[512 + g * 128:512 + (g + 1) * 128, :])
                bp_row = wkv.tile([1, C], F32, tag="bp_row", name="bp_row")
                bp_row_r = wkv.tile([1, C], F32R, tag="bp_row_r",
                                    name="bp_row_r")
                nc.sync.dma_start(bp_row[:], bp_d[None, :])
                nc.vector.tensor_copy(bp_row_r[:], bp_row[:])
                pb = big_tile("pb")
                nc.tensor.matmul(pb[:, 0:HF], ones_row[:], bp_row_r[:, 0:HF],
                                 start=True, stop=True)
                nc.tensor.matmul(pb[:, HF:C], ones_row[:], bp_row_r[:, HF:C],
                                 start=True, stop=True)
                nc.scalar.copy(bias_bc[:], pb[:, 0:C])

            for sc in range(NSC):
                n0 = sc * SC
                kTt = xtp.tile([128, CC, SC], BF16, tag="xT", name="kTt")
                nc.gpsimd.dma_start(
                    kTt[:],
                    kt_d[:, n0:n0 + SC].rearrange("(cc p) n -> p cc n", p=128))
                if sc == 0:
                    # WvT queued after the first k staging chunk so the first
                    # k-projection matmuls start ~5us earlier.
                    load_wT_grouped(WvT, wvt_d)
                in_proj(WkT, kTt, khT_a, khT_b, n0)
                vTt = xtp.tile([128, CC, SC], BF16, tag="xT", name="vTt")
                nc.gpsimd.dma_start(
                    vTt[:],
                    vt_d[:, n0:n0 + SC].rearrange("(cc p) n -> p cc n", p=128))
                for ntl in range(SC // 128):
                    nt = sc * (SC // 128) + ntl
                    p = big_tile("pv")
                    for cc in range(CC):
                        xw = vTt[:, cc, ntl * 128:(ntl + 1) * 128]
                        nc.tensor.matmul(p[:, 0:HF], xw, WvT[:, cc, 0:HF],
                                         start=(cc == 0), stop=(cc == CC - 1))
                        nc.tensor.matmul(p[:, HF:C], xw, WvT[:, cc, HF:C],
                                         start=(cc == 0), stop=(cc == CC - 1))
                    nc.vector.tensor_copy(
                        vh[nt][:, :, 0:DH],
                        p[:, 0:C].rearrange("p (h d) -> p h d", h=H))
                    nc.vector.tensor_copy(
                        vh[nt][:, :, DH:DH + 1],
                        ones_col[:].rearrange("p (h o) -> p h o", h=H))
                if sc == 0:
                    load_wq()
                    load_wp_bias()

        # ---- phase 2: per-superchunk attention + output projection --------
        qhp = ctx.enter_context(tc.tile_pool(name="qhp", bufs=1))
        esp = ctx.enter_context(tc.tile_pool(name="esp", bufs=3))
        xop = ctx.enter_context(tc.tile_pool(name="xop", bufs=1))
        scp = ctx.enter_context(tc.tile_pool(name="scp", bufs=2))
        yp = ctx.enter_context(tc.tile_pool(name="yp", bufs=2))
        bcp = ctx.enter_context(tc.tile_pool(name="bcp", bufs=1))

        qhT_a = [qhp.tile([128, SC], BF16, tag=f"qha{h}", name=f"qha{h}")
                 for h in range(H)]
        qhT_b = [qhp.tile([128, SC], BF16, tag=f"qhb{g}", name=f"qhb{g}")
                 for g in range(2)]
        # raw (unnormalized) U, f32 to keep a single bf16 rounding on x
        ur_a = [xop.tile([128, SC], F32, tag=f"ura{h}", name=f"ura{h}")
                for h in range(H)]
        ur_b = [xop.tile([128, SC], F32, tag=f"urb{g}", name=f"urb{g}")
                for g in range(2)]
        rs = [xop.tile([1, SC], F32, tag=f"rs{h}", name=f"rs{h}")
              for h in range(H)]
        xT_a = [xop.tile([128, SC], BF16, tag=f"xta{h}", name=f"xta{h}")
                for h in range(H)]
        xT_b = [xop.tile([128, SC], BF16, tag=f"xtb{g}", name=f"xtb{g}")
                for g in range(2)]

        def q_load(sc):
            n0 = sc * SC
            qTt = xtp.tile([128, CC, SC], BF16, tag="xT", name="qTt")
            nc.gpsimd.dma_start(
                qTt[:],
                qt_d[:, n0:n0 + SC].rearrange("(cc p) n -> p cc n", p=128))
            return qTt

        def finalize_recip(fh, src=None):
            # 1-partition RECIPROCAL of the rowsum row on DVE (iterative
            # divide, ~6.7us at 1024 wide) - slow but fully hidden under the
            # next head's score loop / the output projection.
            recip = scp.tile([1, SC], F32R, tag="recip", name="recip")
            with nc.allow_low_precision(reason="softmax denom recip f32r"):
                nc.vector.reciprocal(recip[:],
                                     rs[fh][:] if src is None else src)
            return recip

        def finalize_muls(fh, recip):
            # broadcast 1/rowsum across partitions with a rank-1 ones matmul,
            # then normalize the raw U tiles into the bf16 stationaries for
            # the output projection.
            g, blo = fh // 2, (fh % 2) * 64
            bc_ps = big_tile("bcp")
            nc.tensor.matmul(bc_ps[:, 0:HF], ones_row[:], recip[:, 0:HF],
                             start=True, stop=True)
            nc.tensor.matmul(bc_ps[:, HF:SC], ones_row[:], recip[:, HF:SC],
                             start=True, stop=True)
            bc = scp.tile([128, SC], F32, tag="bc", name="bc")
            nc.scalar.copy(bc[:], bc_ps[:])
            nc.vector.tensor_mul(xT_a[fh][:], ur_a[fh][:], bc[:])
            nc.vector.tensor_mul(xT_b[g][blo:blo + 64, :],
                                 ur_b[g][blo:blo + 64, :], bc[blo:blo + 64, :])

        def attention(sc):
            # (h, kt) flattened with the score stream leading the av stream
            # by two groups, so the next head's first exps are already in
            # flight when its avs begin - no per-head-boundary PE gap.
            def scores(h, kt):
                g, blo = h // 2, (h % 2) * 64
                s = big_tile("s")
                wa = khT_a[h][:, kt * 128:(kt + 1) * 128]
                qa = qhT_a[h]
                nc.tensor.matmul(s[:, 0:HF], wa, qa[:, 0:HF],
                                 start=True, stop=False)
                nc.tensor.matmul(s[:, HF:SC], wa, qa[:, HF:SC],
                                 start=True, stop=False)
                wb = khT_b[g][blo:blo + 64, kt * 128:(kt + 1) * 128]
                qb = qhT_b[g]
                nc.tensor.matmul(s[:, 0:HF], wb, qb[blo:blo + 64, 0:HF],
                                 start=False, stop=True)
                nc.tensor.matmul(s[:, HF:SC], wb, qb[blo:blo + 64, HF:SC],
                                 start=False, stop=True)
                es = esp.tile([128, SC], BF16, tag="es", name="es")
                nc.scalar.activation(es[:], s[:], AF.Exp, scale=SCALE)
                return es

            def av(h, kt, u_a, u_b, es):
                va = vh[kt][:, h, 0:128]
                st, sp = (kt == 0), (kt == KT - 1)
                nc.tensor.matmul(u_a[:, 0:HF], va, es[:, 0:HF],
                                 start=st, stop=sp)
                nc.tensor.matmul(u_a[:, HF:SC], va, es[:, HF:SC],
                                 start=st, stop=sp)
                vb = vh[kt][:, h, 128:DH + 1]
                nc.tensor.matmul(u_b[:, 0:HF], vb, es[:, 0:HF],
                                 start=st, stop=sp)
                nc.tensor.matmul(u_b[:, HF:SC], vb, es[:, HF:SC],
                                 start=st, stop=sp)

            seq = [(h, kt) for h in range(H) for kt in range(KT)]
            es_q = [scores(*seq[0]), scores(*seq[1])]
            pend = None
            u = None
            for j, (h, kt) in enumerate(seq):
                if j + 2 < len(seq):
                    es_q.append(scores(*seq[j + 2]))
                if kt == 0:
                    u = (ps.tile([128, SC], F32, tag="ua", name="ua", bufs=1),
                         ps.tile([65, SC], F32, tag="ub", name="ub", bufs=1))
                av(h, kt, *u, es_q.pop(0))
                if kt == 2 and pend is not None:
                    pend = (pend[0], finalize_recip(pend[0]))
                elif kt == 8 and pend is not None:
                    finalize_muls(*pend)
                    pend = None
                elif kt == KT - 1:
                    # evacuate raw U immediately so the single-buffered U
                    # psum frees before the next head's first av matmul.
                    # The rowsum row stays in psum for the LAST head: its
                    # reciprocal runs during the projection phase, when no
                    # next av needs the ub buffer, and skipping the staging
                    # copy shortens the normalization chain.
                    g, blo = h // 2, (h % 2) * 64
                    nc.scalar.copy(ur_a[h][:], u[0][:])
                    nc.vector.tensor_copy(ur_b[g][blo:blo + 64, :],
                                          u[1][0:64, :])
                    if h != H - 1:
                        nc.vector.tensor_copy(rs[h][:], u[1][64:65, :])
                    pend = (h,)
            return pend[0], u[1]

        def q_proj_groups(qTt, scalar_evac=True):
            # one projection group per output j-chunk; emitted lazily so the
            # caller can interleave them with the output projection. PSUM
            # evacuation on ScalarE when these groups run while the pending
            # head's 6.5us RECIPROCAL occupies the DVE FIFO (a DVE evac
            # there would stall the PE's psum-buffer rotation behind it);
            # on DVE for the pre-loop instance, where ScalarE copies would
            # instead delay the first attention exps.
            def group(jc):
                p = big_tile("pq")
                for cc in range(CC):
                    w = WqT[:, cc, jc * 128:(jc + 1) * 128]
                    nc.tensor.matmul(p[:, 0:HF], w, qTt[:, cc, 0:HF],
                                     start=(cc == 0), stop=(cc == CC - 1))
                    nc.tensor.matmul(p[:, HF:SC], w, qTt[:, cc, HF:SC],
                                     start=(cc == 0), stop=(cc == CC - 1))
                cp = nc.scalar.copy if scalar_evac else nc.vector.tensor_copy
                for (plo, phi, kind, idx, dlo) in _jc_segments(jc):
                    cp(seg_dest(kind, idx, dlo, dlo + (phi - plo),
                                qhT_a, qhT_b, 0, SC),
                       p[plo:phi, :])
            return [lambda jc=jc: group(jc) for jc in range(CC)]

        def final_part(p, ntl, heads, start, stop):
            for h in heads:
                g, blo = h // 2, (h % 2) * 64
                xa = xT_a[h][:, ntl * 128:(ntl + 1) * 128]
                nc.tensor.matmul(p[:, 0:HF], xa, WpT_a[:, h, 0:HF],
                                 start=start and h == heads[0], stop=False)
                nc.tensor.matmul(p[:, HF:C], xa, WpT_a[:, h, HF:C],
                                 start=start and h == heads[0], stop=False)
                xb = xT_b[g][blo:blo + 64, ntl * 128:(ntl + 1) * 128]
                nc.tensor.matmul(p[:, 0:HF], xb, WpT_b[g][blo:blo + 64, 0:HF],
                                 start=False, stop=stop and h == heads[-1])
                nc.tensor.matmul(p[:, HF:C], xb, WpT_b[g][blo:blo + 64, HF:C],
                                 start=False, stop=stop and h == heads[-1])

        def final_evac(p, sc, ntl):
            n0 = sc * SC
            ysb = yp.tile([128, C], F32, tag="y", name="y")
            nc.vector.tensor_add(ysb[:], p[:, 0:C], bias_bc[:])
            nc.sync.dma_start(
                y_d[n0 + ntl * 128:n0 + (ntl + 1) * 128, :], ysb[:])

        def final_ntl(sc, ntl):
            p = big_tile("py")
            final_part(p, ntl, list(range(H)), True, True)
            final_evac(p, sc, ntl)

        def finalize_tail(fh, ub_last):
            # no cover work exists after the last superchunk's attention, so
            # split the pending head's normalization into column quarters -
            # final_ntl(ntl) only needs xT columns [ntl*128,(ntl+1)*128), so
            # the output projection starts after a quarter of the reciprocal,
            # read straight from the rowsum row still sitting in psum.
            # The broadcast psum rides the now-idle "ua" bank pair: the "big"
            # bufs hold the two open part_a groups (deadlock otherwise).
            g, blo = fh // 2, (fh % 2) * 64
            recip = scp.tile([1, SC], F32R, tag="recip", name="recip")
            bc = bcp.tile([128, SC], F32R, tag="bcr", name="bcr")
            for pi in range(4):
                lo, hi = pi * (SC // 4), (pi + 1) * (SC // 4)
                with nc.allow_low_precision(reason="softmax denom recip"):
                    nc.vector.reciprocal(recip[:, lo:hi],
                                         ub_last[64:65, lo:hi])
                    # partition broadcast on the otherwise-idle GpSimd DMA
                    # engine: no psum bank, no PE matmul, no ScalarE copy in
                    # the chain, so the projection cover never stalls.
                    nc.gpsimd.partition_broadcast(bc[:, lo:hi],
                                                  recip[:, lo:hi])
                    nc.vector.tensor_mul(xT_a[fh][:, lo:hi],
                                         ur_a[fh][:, lo:hi], bc[:, lo:hi])
                    nc.vector.tensor_mul(xT_b[g][blo:blo + 64, lo:hi],
                                         ur_b[g][blo:blo + 64, lo:hi],
                                         bc[blo:blo + 64, lo:hi])

        # q-superchunk pipeline: attention(sc) leaves the last head's
        # normalization pending; the next superchunk's q-projection groups
        # are PE work independent of it and cover the reciprocal chain, then
        # the output projection consumes the normalized stationaries.
        qTt = q_load(0)
        for fn in q_proj_groups(qTt, scalar_evac=False):
            fn()
        for sc in range(NSC):
            if sc + 1 < NSC:
                qt_next = q_load(sc + 1)
            fh, ub_last = attention(sc)
            if sc + 1 < NSC:
                recip = finalize_recip(fh, src=ub_last[64:65, :])
                qp = q_proj_groups(qt_next)
                for fn in qp[0:4]:
                    fn()
                finalize_muls(fh, recip)
                rest = qp[4:]
                for ntl in range(SC // 128):
                    final_ntl(sc, ntl)
                    if rest:
                        rest.pop(0)()
            else:
                # tail: open the first two output-projection groups with the
                # already-normalized heads as cover work, weave the pending
                # head's quartered normalization in, then close them.
                heads_a = [h for h in range(H) if h != fh]
                p0 = big_tile("py")
                final_part(p0, 0, heads_a, True, False)
                p1 = big_tile("py")
                final_part(p1, 1, heads_a, True, False)
                finalize_tail(fh, ub_last)
                final_part(p0, 0, [fh], False, True)
                final_evac(p0, sc, 0)
                final_part(p1, 1, [fh], False, True)
                final_evac(p1, sc, 1)
                for ntl in range(2, SC // 128):
                    final_ntl(sc, ntl)

    nc.compile()
    return nc


def _get_built():
    global _BUILT
    if _BUILT is None:
        _BUILT = _build()
    return _BUILT


def run(inputs, trace=False, **kw):
    """Run on all 8 cores; returns (y [B,N,C] float32, BassKernelResults)."""
    import ml_dtypes
    from concourse.bass_utils import run_bass_kernel_spmd

    nc = _get_built()
    bf16 = ml_dtypes.bfloat16
    f32 = np.float32
    wpt = np.asarray(inputs["Wp"], f32).T  # [c', j]
    wpt_packed = np.concatenate(
        [wpt[h * DH:h * DH + 128] for h in range(H)]
        + [wpt[h * DH + 128:(h + 1) * DH] for h in range(H)])
    shared = {
        "WqT": np.ascontiguousarray(np.asarray(inputs["Wq"], f32).T).astype(bf16),
        "WkT": np.ascontiguousarray(np.asarray(inputs["Wk"], f32).T).astype(bf16),
        "WvT": np.ascontiguousarray(np.asarray(inputs["Wv"], f32).T).astype(bf16),
        "WpT": np.ascontiguousarray(wpt_packed).astype(bf16),
        "bp": np.ascontiguousarray(np.asarray(inputs["bp"], f32)),
    }
    q = np.asarray(inputs["q"], f32)
    k = np.asarray(inputs["k"], f32)
    v = np.asarray(inputs["v"], f32)
    in_maps = []
    for b in range(B):
        m = dict(shared)
        m["qT"] = np.ascontiguousarray(q[b].T).astype(bf16)
        m["kT"] = np.ascontiguousarray(k[b].T).astype(bf16)
        m["vT"] = np.ascontiguousarray(v[b].T).astype(bf16)
        in_maps.append(m)
    res = run_bass_kernel_spmd(nc, in_maps, list(range(B)), trace=trace, **kw)
    y = np.stack([res.results[b]["y"] for b in range(B)]).astype(np.float32)
    return y, res


def kernel(q, k, v, Wq, Wk, Wv, Wp, bp):
    y, _ = run({"q": q, "k": k, "v": v, "Wq": Wq, "Wk": Wk, "Wv": Wv,
                "Wp": Wp, "bp": bp})
    return y


# revision 23
# speedup vs baseline: 1.2829x; 1.0686x over previous
"""Trainium2 Bass kernel for nn_Attention_44994077393310.

Multi-head attention (B=8, N=2048, C=768, H=4, Dh=192) with input projections,
softmax attention, and output projection with bias.

Sharding: pure data-parallel over the batch dim - each of the 8 NeuronCores
computes one batch element end-to-end (weights replicated). No collectives.

v2: all matmul operands are bf16 (inputs cast on the host). Rationale from the
v1 (fp32r) trace: the PE was 93% busy but ~1/3 of its time was exposed
LDWEIGHTS - fp32 weights disable the HW fast-weight-load path and every
512-col matmul serialized a ~110ns weight load. bf16 enables FWL, halves DMA
bytes, and the kernel is restructured so consecutive matmul pairs share one
stationary operand (1024-wide q superchunks processed as two 512-col moving
halves per weight load), halving the LDWEIGHTS count.

Layout strategy (unchanged from v1): q/k/v and weights are pre-transposed on
the host so every DMA lands operands with the contraction dim on partitions.
Scores are computed transposed S_T[key, q] with exp on ScalarE (scale folded
in); a ones column in vh makes softmax denominators fall out of the U = es@v
matmuls; U is evacuated RAW to SBUF (f32) as soon as a head finishes so the
single-buffered U psum frees immediately, and the slow 1-partition RECIPROCAL
+ broadcast + normalize runs on DVE hidden under the next head / the output
projection.

PSUM plan (8 banks): one pool - tag "big" [128,1024]f32 x2 bufs (4 banks,
used by scores/projections/bc broadcast), tags "ua" [128,1024] + "ub"
[65,1024] x1 buf (4 banks, the per-head U accumulators).
"""

import numpy as np

B = 8
N = 2048
C = 768
H = 4
DH = 192
SCALE = DH ** -0.5

NSC = 2                 # superchunks of 1024 over the sequence
SC = N // NSC           # 1024
HF = SC // 2            # 512 (moving-operand half width)
CC = C // 128           # 6 channel chunks
KT = N // 128           # 16 k-tiles
NWARM = 44

_BUILT = None


def _dest_of(cp):
    h, dd = divmod(cp, DH)
    if dd < 128:
        return ("a", h, dd)
    return ("b", h // 2, (h % 2) * 64 + (dd - 128))


def _jc_segments(jc):
    """Merged PSUM->head-major copy segments for projection j-chunk jc."""
    segs = []
    for p0 in range(0, 128, 64):
        kind, idx, dlo = _dest_of(128 * jc + p0)
        if segs and segs[-1][2] == kind and segs[-1][3] == idx and \
                segs[-1][4] + (segs[-1][1] - segs[-1][0]) == dlo:
            segs[-1] = (segs[-1][0], p0 + 64, kind, idx, segs[-1][4])
        else:
            segs.append((p0, p0 + 64, kind, idx, dlo))
    return segs


def _build():
    from contextlib import ExitStack

    import concourse.mybir as mybir
    import concourse.tile as tile
    from concourse import bacc

    F32 = mybir.dt.float32
    F32R = mybir.dt.float32r
    BF16 = mybir.dt.bfloat16
    AF = mybir.ActivationFunctionType

    nc = bacc.Bacc("TRN2", target_bir_lowering=False, debug=False)
    qt_d = nc.dram_tensor("qT", [C, N], BF16, kind="ExternalInput").ap()
    kt_d = nc.dram_tensor("kT", [C, N], BF16, kind="ExternalInput").ap()
    vt_d = nc.dram_tensor("vT", [C, N], BF16, kind="ExternalInput").ap()
    wqt_d = nc.dram_tensor("WqT", [C, C], BF16, kind="ExternalInput").ap()
    wkt_d = nc.dram_tensor("WkT", [C, C], BF16, kind="ExternalInput").ap()
    wvt_d = nc.dram_tensor("WvT", [C, C], BF16, kind="ExternalInput").ap()
    wpt_d = nc.dram_tensor("WpT", [C, C], BF16, kind="ExternalInput").ap()
    bp_d = nc.dram_tensor("bp", [C], F32, kind="ExternalInput").ap()
    y_d = nc.dram_tensor("y", [N, C], F32, kind="ExternalOutput").ap()

    with tile.TileContext(nc) as tc, ExitStack() as ctx:
        const = ctx.enter_context(tc.tile_pool(name="const", bufs=1))
        wqp = ctx.enter_context(tc.tile_pool(name="wqp", bufs=1))
        khp = ctx.enter_context(tc.tile_pool(name="khp", bufs=1))
        vhp = ctx.enter_context(tc.tile_pool(name="vhp", bufs=1))
        xtp = ctx.enter_context(tc.tile_pool(name="xtp", bufs=2))
        ps = ctx.enter_context(tc.tile_pool(name="ps", bufs=2, space="PSUM"))

        def big_tile(nm):
            return ps.tile([128, SC], F32, tag="big", name=nm, bufs=2)

        ones_col = const.tile([128, H], BF16, tag="ones_col", name="ones_col")
        nc.vector.memset(ones_col[:], 1.0)
        ones_row_f = const.tile([1, 128], F32, tag="ones_row_f",
                                name="ones_row_f")
        nc.vector.memset(ones_row_f[:], 1.0)
        ones_row = const.tile([1, 128], F32R, tag="ones_row", name="ones_row")
        nc.vector.tensor_copy(ones_row[:], ones_row_f[:])

        # PE warm-up: dependency-free matmuls so the HAM clock gate opens
        # while the first DMAs stream in.
        warm_w_f = const.tile([128, 128], F32, tag="warm_w_f", name="warm_w_f")
        nc.vector.memset(warm_w_f[:], 0.5)
        warm_w = const.tile([128, 128], BF16, tag="warm_w", name="warm_w")
        nc.vector.tensor_copy(warm_w[:], warm_w_f[:])
        warm_x = const.tile([128, HF], BF16, tag="warm_x", name="warm_x")
        for i in range(4):
            nc.vector.tensor_copy(warm_x[:, i * 128:(i + 1) * 128], warm_w_f[:])
        for r in range(NWARM):
            wp = ps.tile([128, SC], F32, tag="ua", name="warm_ps", bufs=1)
            nc.tensor.matmul(wp[:, 0:HF], warm_w[:], warm_x[:],
                             start=True, stop=True)

        # ---- persistent weights (direct loads, no transposes) -------------
        WqT = wqp.tile([128, CC, C], BF16, tag="wqt", name="wqt")
        WpT_a = wqp.tile([128, H, C], BF16, tag="wpa", name="wpa")
        WpT_b = [wqp.tile([128, C], BF16, tag=f"wpb{g}", name=f"wpb{g}")
                 for g in range(2)]
        bias_bc = wqp.tile([128, C], F32, tag="bias_bc", name="bias_bc")

        khT_a = [khp.tile([128, N], BF16, tag=f"kha{h}", name=f"kha{h}")
                 for h in range(H)]
        khT_b = [khp.tile([128, N], BF16, tag=f"khb{g}", name=f"khb{g}")
                 for g in range(2)]
        vh = [vhp.tile([128, H, DH + 1], BF16, tag=f"vh{nt}", name=f"vh{nt}")
              for nt in range(KT)]

        def load_wT_grouped(dest, w_dram):
            # dest[p, cc, j] = W.T[cc*128+p, j]
            nc.gpsimd.dma_start(
                dest[:],
                w_dram.rearrange("(cc p) j -> p cc j", p=128))

        def seg_dest(kind, idx, dlo, dhi, a_tiles, b_tiles, col_lo, col_hi):
            t = a_tiles[idx] if kind == "a" else b_tiles[idx]
            return t[dlo:dhi, col_lo:col_hi]

        def in_proj(w_tiles, xTt, a_tiles, b_tiles, n0):
            # out.T[j, n0:n0+SC] head-major packed; one weight load feeds the
            # two 512-col moving halves.
            for jc in range(CC):
                p = big_tile("pj")
                for cc in range(CC):
                    w = w_tiles[:, cc, jc * 128:(jc + 1) * 128]
                    nc.tensor.matmul(p[:, 0:HF], w, xTt[:, cc, 0:HF],
                                     start=(cc == 0), stop=(cc == CC - 1))
                    nc.tensor.matmul(p[:, HF:SC], w, xTt[:, cc, HF:SC],
                                     start=(cc == 0), stop=(cc == CC - 1))
                for (plo, phi, kind, idx, dlo) in _jc_segments(jc):
                    nc.vector.tensor_copy(
                        seg_dest(kind, idx, dlo, dlo + (phi - plo),
                                 a_tiles, b_tiles, n0, n0 + SC),
                        p[plo:phi, :])

        # ---- phase 1: stage k, v ------------------------------------------
        with tc.tile_pool(name="wkv", bufs=1) as wkv:
            WkT = wkv.tile([128, CC, C], BF16, tag="wkt", name="wkt")
            WvT = wkv.tile([128, CC, C], BF16, tag="wvt", name="wvt")
            load_wT_grouped(WkT, wkt_d)

            def load_wq():
                load_wT_grouped(WqT, wqt_d)

            def load_wp_bias():
                # wpt_d is host-packed head-major: rows 0..511 = per-head
                # dd 0..127 (h-major), rows 512..639 / 640..767 = the packed
                # b-tiles (dd 128..191 of heads 0,1 / 2,3).
                nc.gpsimd.dma_start(
                    WpT_a[:],
                    wpt_d[0:512, :].rearrange("(h p) j -> p h j", p=128))
                for g in range(2):
                    nc.gpsimd.dma_start(
                        WpT_b[g][:], wpt_d[512 + g * 128:512 + (g + 1) * 128, :])
                bp_row = wkv.tile([1, C], F32, tag="bp_row", name="bp_row")
                bp_row_r = wkv.tile([1, C], F32R, tag="bp_row_r",
                                    name="bp_row_r")
                nc.sync.dma_start(bp_row[:], bp_d[None, :])
                nc.vector.tensor_copy(bp_row_r[:], bp_row[:])
                pb = big_tile("pb")
                nc.tensor.matmul(pb[:, 0:HF], ones_row[:], bp_row_r[:, 0:HF],
                                 start=True, stop=True)
                nc.tensor.matmul(pb[:, HF:C], ones_row[:], bp_row_r[:, HF:C],
                                 start=True, stop=True)
                nc.scalar.copy(bias_bc[:], pb[:, 0:C])

            for sc in range(NSC):
                n0 = sc * SC
                kTt = xtp.tile([128, CC, SC], BF16, tag="xT", name="kTt")
                nc.gpsimd.dma_start(
                    kTt[:],
                    kt_d[:, n0:n0 + SC].rearrange("(cc p) n -> p cc n", p=128))
                if sc == 0:
                    # WvT queued after the first k staging chunk so the first
                    # k-projection matmuls start ~5us earlier.
                    load_wT_grouped(WvT, wvt_d)
                in_proj(WkT, kTt, khT_a, khT_b, n0)
                vTt = xtp.tile([128, CC, SC], BF16, tag="xT", name="vTt")
                nc.gpsimd.dma_start(
                    vTt[:],
                    vt_d[:, n0:n0 + SC].rearrange("(cc p) n -> p cc n", p=128))
                for ntl in range(SC // 128):
                    nt = sc * (SC // 128) + ntl
                    p = big_tile("pv")
                    for cc in range(CC):
                        xw = vTt[:, cc, ntl * 128:(ntl + 1) * 128]
                        nc.tensor.matmul(p[:, 0:HF], xw, WvT[:, cc, 0:HF],
                                         start=(cc == 0), stop=(cc == CC - 1))
                        nc.tensor.matmul(p[:, HF:C], xw, WvT[:, cc, HF:C],
                                         start=(cc == 0), stop=(cc == CC - 1))
                    nc.vector.tensor_copy(
                        vh[nt][:, :, 0:DH],
                        p[:, 0:C].rearrange("p (h d) -> p h d", h=H))
                    nc.vector.tensor_copy(
                        vh[nt][:, :, DH:DH + 1],
                        ones_col[:].rearrange("p (h o) -> p h o", h=H))
                if sc == 0:
                    load_wq()
                    load_wp_bias()

        # ---- phase 2: per-superchunk attention + output projection --------
        qhp = ctx.enter_context(tc.tile_pool(name="qhp", bufs=1))
        esp = ctx.enter_context(tc.tile_pool(name="esp", bufs=3))
        xop = ctx.enter_context(tc.tile_pool(name="xop", bufs=1))
        scp = ctx.enter_context(tc.tile_pool(name="scp", bufs=2))
        yp = ctx.enter_context(tc.tile_pool(name="yp", bufs=2))
        bcp = ctx.enter_context(tc.tile_pool(name="bcp", bufs=1))

        qhT_a = [qhp.tile([128, SC], BF16, tag=f"qha{h}", name=f"qha{h}")
                 for h in range(H)]
        qhT_b = [qhp.tile([128, SC], BF16, tag=f"qhb{g}", name=f"qhb{g}")
                 for g in range(2)]
        # raw (unnormalized) U, f32 to keep a single bf16 rounding on x
        ur_a = [xop.tile([128, SC], F32, tag=f"ura{h}", name=f"ura{h}")
                for h in range(H)]
        ur_b = [xop.tile([128, SC], F32, tag=f"urb{g}", name=f"urb{g}")
                for g in range(2)]
        rs = [xop.tile([1, SC], F32, tag=f"rs{h}", name=f"rs{h}")
              for h in range(H)]
        xT_a = [xop.tile([128, SC], BF16, tag=f"xta{h}", name=f"xta{h}")
                for h in range(H)]
        xT_b = [xop.tile([128, SC], BF16, tag=f"xtb{g}", name=f"xtb{g}")
                for g in range(2)]

        def q_load(sc):
            n0 = sc * SC
            qTt = xtp.tile([128, CC, SC], BF16, tag="xT", name="qTt")
            nc.gpsimd.dma_start(
                qTt[:],
                qt_d[:, n0:n0 + SC].rearrange("(cc p) n -> p cc n", p=128))
            return qTt

        def finalize_recip(fh, src=None):
            # 1-partition RECIPROCAL of the rowsum row on DVE (iterative
            # divide, ~6.7us at 1024 wide) - slow but fully hidden under the
            # next head's score loop / the output projection.
            recip = scp.tile([1, SC], F32R, tag="recip", name="recip")
            with nc.allow_low_precision(reason="softmax denom recip f32r"):
                nc.vector.reciprocal(recip[:],
                                     rs[fh][:] if src is None else src)
            return recip

        def finalize_muls(fh, recip):
            # broadcast 1/rowsum across partitions with a rank-1 ones matmul,
            # then normalize the raw U tiles into the bf16 stationaries for
            # the output projection.
            g, blo = fh // 2, (fh % 2) * 64
            bc_ps = big_tile("bcp")
            nc.tensor.matmul(bc_ps[:, 0:HF], ones_row[:], recip[:, 0:HF],
                             start=True, stop=True)
            nc.tensor.matmul(bc_ps[:, HF:SC], ones_row[:], recip[:, HF:SC],
                             start=True, stop=True)
            bc = scp.tile([128, SC], F32, tag="bc", name="bc")
            nc.scalar.copy(bc[:], bc_ps[:])
            nc.vector.tensor_mul(xT_a[fh][:], ur_a[fh][:], bc[:])
            nc.vector.tensor_mul(xT_b[g][blo:blo + 64, :],
                                 ur_b[g][blo:blo + 64, :], bc[blo:blo + 64, :])

        def attention(sc):
            # (h, kt) flattened with the score stream leading the av stream
            # by two groups, so the next head's first exps are already in
            # flight when its avs begin - no per-head-boundary PE gap.
            def scores(h, kt):
                g, blo = h // 2, (h % 2) * 64
                s = big_tile("s")
                wa = khT_a[h][:, kt * 128:(kt + 1) * 128]
                qa = qhT_a[h]
                nc.tensor.matmul(s[:, 0:HF], wa, qa[:, 0:HF],
                                 start=True, stop=False)
                nc.tensor.matmul(s[:, HF:SC], wa, qa[:, HF:SC],
                                 start=True, stop=False)
                wb = khT_b[g][blo:blo + 64, kt * 128:(kt + 1) * 128]
                qb = qhT_b[g]
                nc.tensor.matmul(s[:, 0:HF], wb, qb[blo:blo + 64, 0:HF],
                                 start=False, stop=True)
                nc.tensor.matmul(s[:, HF:SC], wb, qb[blo:blo + 64, HF:SC],
                                 start=False, stop=True)
                es = esp.tile([128, SC], BF16, tag="es", name="es")
                nc.scalar.activation(es[:], s[:], AF.Exp, scale=SCALE)
                return es

            def av(h, kt, u_a, u_b, es):
                va = vh[kt][:, h, 0:128]
                st, sp = (kt == 0), (kt == KT - 1)
                nc.tensor.matmul(u_a[:, 0:HF], va, es[:, 0:HF],
                                 start=st, stop=sp)
                nc.tensor.matmul(u_a[:, HF:SC], va, es[:, HF:SC],
                                 start=st, stop=sp)
                vb = vh[kt][:, h, 128:DH + 1]
                nc.tensor.matmul(u_b[:, 0:HF], vb, es[:, 0:HF],
                                 start=st, stop=sp)
                nc.tensor.matmul(u_b[:, HF:SC], vb, es[:, HF:SC],
                                 start=st, stop=sp)

            seq = [(h, kt) for h in range(H) for kt in range(KT)]
            es_q = [scores(*seq[0]), scores(*seq[1])]
            pend = None
            u = None
            for j, (h, kt) in enumerate(seq):
                if j + 2 < len(seq):
                    es_q.append(scores(*seq[j + 2]))
                if kt == 0:
                    u = (ps.tile([128, SC], F32, tag="ua", name="ua", bufs=1),
                         ps.tile([65, SC], F32, tag="ub", name="ub", bufs=1))
                av(h, kt, *u, es_q.pop(0))
                if kt == 2 and pend is not None:
                    pend = (pend[0], finalize_recip(pend[0]))
                elif kt == 8 and pend is not None:
                    finalize_muls(*pend)
                    pend = None
                elif kt == KT - 1:
                    # evacuate raw U immediately so the single-buffered U
                    # psum frees before the next head's first av matmul.
                    # The rowsum row stays in psum for the LAST head: its
                    # reciprocal runs during the projection phase, when no
                    # next av needs the ub buffer, and skipping the staging
                    # copy shortens the normalization chain.
                    g, blo = h // 2, (h % 2) * 64
                    nc.scalar.copy(ur_a[h][:], u[0][:])
                    nc.vector.tensor_copy(ur_b[g][blo:blo + 64, :],
                                          u[1][0:64, :])
                    if h != H - 1:
                        nc.vector.tensor_copy(rs[h][:], u[1][64:65, :])
                    pend = (h,)
            return pend[0], u[1]

        def q_proj_groups(qTt, scalar_evac=True):
            # one projection group per output j-chunk; emitted lazily so the
            # caller can interleave them with the output projection. PSUM
            # evacuation on ScalarE when these groups run while the pending
            # head's 6.5us RECIPROCAL occupies the DVE FIFO (a DVE evac
            # there would stall the PE's psum-buffer rotation behind it);
            # on DVE for the pre-loop instance, where ScalarE copies would
            # instead delay the first attention exps.
            def group(jc):
                p = big_tile("pq")
                for cc in range(CC):
                    w = WqT[:, cc, jc * 128:(jc + 1) * 128]
                    nc.tensor.matmul(p[:, 0:HF], w, qTt[:, cc, 0:HF],
                                     start=(cc == 0), stop=(cc == CC - 1))
                    nc.tensor.matmul(p[:, HF:SC], w, qTt[:, cc, HF:SC],
                                     start=(cc == 0), stop=(cc == CC - 1))
                cp = nc.scalar.copy if scalar_evac else nc.vector.tensor_copy
                for (plo, phi, kind, idx, dlo) in _jc_segments(jc):
                    cp(seg_dest(kind, idx, dlo, dlo + (phi - plo),
                                qhT_a, qhT_b, 0, SC),
                       p[plo:phi, :])
            return [lambda jc=jc: group(jc) for jc in range(CC)]

        def final_part(p, ntl, heads, start, stop):
            # the two heads of a pair share xT_b[g]/WpT_b[g] partition
            # packing, and their b contributions sum into the same output -
            # one full-K=128 matmul replaces two 64-K ones when both heads
            # are present (half the streamed columns).
            ops = [("a", h) for h in heads]
            for g in range(2):
                pair = [h for h in heads if h // 2 == g]
                if len(pair) == 2:
                    ops.append(("b2", g))
                elif pair:
                    ops.append(("b1", pair[0]))
            for i, (kind, v) in enumerate(ops):
                st = start and i == 0
                sp = stop and i == len(ops) - 1
                if kind == "a":
                    w = xT_a[v][:, ntl * 128:(ntl + 1) * 128]
                    ra, rb = WpT_a[:, v, 0:HF], WpT_a[:, v, HF:C]
                elif kind == "b2":
                    w = xT_b[v][:, ntl * 128:(ntl + 1) * 128]
                    ra, rb = WpT_b[v][:, 0:HF], WpT_b[v][:, HF:C]
                else:
                    g, blo = v // 2, (v % 2) * 64
                    w = xT_b[g][blo:blo + 64, ntl * 128:(ntl + 1) * 128]
                    ra = WpT_b[g][blo:blo + 64, 0:HF]
                    rb = WpT_b[g][blo:blo + 64, HF:C]
                nc.tensor.matmul(p[:, 0:HF], w, ra, start=st, stop=sp)
                nc.tensor.matmul(p[:, HF:C], w, rb, start=st, stop=sp)

        def final_evac(p, sc, ntl):
            n0 = sc * SC
            ysb = yp.tile([128, C], F32, tag="y", name="y")
            nc.vector.tensor_add(ysb[:], p[:, 0:C], bias_bc[:])
            nc.sync.dma_start(
                y_d[n0 + ntl * 128:n0 + (ntl + 1) * 128, :], ysb[:])

        def final_ntl(sc, ntl):
            p = big_tile("py")
            final_part(p, ntl, list(range(H)), True, True)
            final_evac(p, sc, ntl)

        def finalize_tail(fh, ub_last):
            # no cover work exists after the last superchunk's attention, so
            # split the pending head's normalization into column quarters -
            # final_ntl(ntl) only needs xT columns [ntl*128,(ntl+1)*128), so
            # the output projection starts after a quarter of the reciprocal,
            # read straight from the rowsum row still sitting in psum.
            # The broadcast psum rides the now-idle "ua" bank pair: the "big"
            # bufs hold the two open part_a groups (deadlock otherwise).
            g, blo = fh // 2, (fh % 2) * 64
            recip = scp.tile([1, SC], F32R, tag="recip", name="recip")
            bc = bcp.tile([128, SC], F32R, tag="bcr", name="bcr")
            for pi in range(4):
                lo, hi = pi * (SC // 4), (pi + 1) * (SC // 4)
                with nc.allow_low_precision(reason="softmax denom recip"):
                    nc.vector.reciprocal(recip[:, lo:hi],
                                         ub_last[64:65, lo:hi])
                    # partition broadcast on the otherwise-idle GpSimd DMA
                    # engine: no psum bank, no PE matmul, no ScalarE copy in
                    # the chain, so the projection cover never stalls.
                    nc.gpsimd.partition_broadcast(bc[:, lo:hi],
                                                  recip[:, lo:hi])
                    nc.vector.tensor_mul(xT_a[fh][:, lo:hi],
                                         ur_a[fh][:, lo:hi], bc[:, lo:hi])
                    nc.vector.tensor_mul(xT_b[g][blo:blo + 64, lo:hi],
                                         ur_b[g][blo:blo + 64, lo:hi],
                                         bc[blo:blo + 64, lo:hi])

        # q-superchunk pipeline: attention(sc) leaves the last head's
        # normalization pending; the next superchunk's q-projection groups
        # are PE work independent of it and cover the reciprocal chain, then
        # the output projection consumes the normalized stationaries.
        qTt = q_load(0)
        for fn in q_proj_groups(qTt, scalar_evac=False):
            fn()
        for sc in range(NSC):
            if sc + 1 < NSC:
                qt_next = q_load(sc + 1)
            fh, ub_last = attention(sc)
            if sc + 1 < NSC:
                recip = finalize_recip(fh, src=ub_last[64:65, :])
                qp = q_proj_groups(qt_next)
                for fn in qp[0:4]:
                    fn()
                finalize_muls(fh, recip)
                rest = qp[4:]
                for ntl in range(SC // 128):
                    final_ntl(sc, ntl)
                    if rest:
                        rest.pop(0)()
            else:
                # tail: open the first two output-projection groups with the
                # already-normalized heads as cover work, weave the pending
                # head's quartered normalization in, then close them.
                heads_a = [h for h in range(H) if h != fh]
                p0 = big_tile("py")
                final_part(p0, 0, heads_a, True, False)
                p1 = big_tile("py")
                final_part(p1, 1, heads_a, True, False)
                finalize_tail(fh, ub_last)
                final_part(p0, 0, [fh], False, True)
                final_evac(p0, sc, 0)
                final_part(p1, 1, [fh], False, True)
                final_evac(p1, sc, 1)
                for ntl in range(2, SC // 128):
                    final_ntl(sc, ntl)

    nc.compile()
    return nc


def _get_built():
    global _BUILT
    if _BUILT is None:
        _BUILT = _build()
    return _BUILT


def run(inputs, trace=False, **kw):
    """Run on all 8 cores; returns (y [B,N,C] float32, BassKernelResults)."""
    import ml_dtypes
    from concourse.bass_utils import run_bass_kernel_spmd

    nc = _get_built()
    bf16 = ml_dtypes.bfloat16
    f32 = np.float32
    wpt = np.asarray(inputs["Wp"], f32).T  # [c', j]
    wpt_packed = np.concatenate(
        [wpt[h * DH:h * DH + 128] for h in range(H)]
        + [wpt[h * DH + 128:(h + 1) * DH] for h in range(H)])
    shared = {
        "WqT": np.ascontiguousarray(np.asarray(inputs["Wq"], f32).T).astype(bf16),
        "WkT": np.ascontiguousarray(np.asarray(inputs["Wk"], f32).T).astype(bf16),
        "WvT": np.ascontiguousarray(np.asarray(inputs["Wv"], f32).T).astype(bf16),
        "WpT": np.ascontiguousarray(wpt_packed).astype(bf16),
        "bp": np.ascontiguousarray(np.asarray(inputs["bp"], f32)),
    }
    q = np.asarray(inputs["q"], f32)
    k = np.asarray(inputs["k"], f32)
    v = np.asarray(inputs["v"], f32)
    in_maps = []
    for b in range(B):
        m = dict(shared)
        m["qT"] = np.ascontiguousarray(q[b].T).astype(bf16)
        m["kT"] = np.ascontiguousarray(k[b].T).astype(bf16)
        m["vT"] = np.ascontiguousarray(v[b].T).astype(bf16)
        in_maps.append(m)
    res = run_bass_kernel_spmd(nc, in_maps, list(range(B)), trace=trace, **kw)
    y = np.stack([res.results[b]["y"] for b in range(B)]).astype(np.float32)
    return y, res


def kernel(q, k, v, Wq, Wk, Wv, Wp, bp):
    y, _ = run({"q": q, "k": k, "v": v, "Wq": Wq, "Wk": Wk, "Wv": Wv,
                "Wp": Wp, "bp": bp})
    return y


# revision 25
# speedup vs baseline: 1.2831x; 1.0001x over previous
"""Trainium2 Bass kernel for nn_Attention_44994077393310.

Multi-head attention (B=8, N=2048, C=768, H=4, Dh=192) with input projections,
softmax attention, and output projection with bias.

Sharding: pure data-parallel over the batch dim - each of the 8 NeuronCores
computes one batch element end-to-end (weights replicated). No collectives.

v2: all matmul operands are bf16 (inputs cast on the host). Rationale from the
v1 (fp32r) trace: the PE was 93% busy but ~1/3 of its time was exposed
LDWEIGHTS - fp32 weights disable the HW fast-weight-load path and every
512-col matmul serialized a ~110ns weight load. bf16 enables FWL, halves DMA
bytes, and the kernel is restructured so consecutive matmul pairs share one
stationary operand (1024-wide q superchunks processed as two 512-col moving
halves per weight load): a repeated weight load hides completely under the
previous matmul, so only genuine stationary CHANGES (~95ns each) remain.
The output projection merges each head pair's 64-row b-part matmuls into one
full-K=128 matmul (their contributions sum in the output and xT_b/WpT_b
already pack the pair on complementary partitions), halving those columns.
fp8/DoubleRow was evaluated and rejected: numpy emulation of e4m3 rounding
puts every injection point (qk, v, es) above the 2e-2 error budget.
Measured HW exec ~435-470us (run-to-run device-state variance is ~+-10%;
identical NEFFs measured 435 and 521us back to back).

Layout strategy (unchanged from v1): q/k/v and weights are pre-transposed on
the host so every DMA lands operands with the contraction dim on partitions.
Scores are computed transposed S_T[key, q] with exp on ScalarE (scale folded
in); a ones column in vh makes softmax denominators fall out of the U = es@v
matmuls; U is evacuated RAW to SBUF (f32) as soon as a head finishes so the
single-buffered U psum frees immediately, and the slow 1-partition RECIPROCAL
+ broadcast + normalize runs on DVE hidden under the next head / the output
projection.

PSUM plan (8 banks): one pool - tag "big" [128,1024]f32 x2 bufs (4 banks,
used by scores/projections/bc broadcast), tags "ua" [128,1024] + "ub"
[65,1024] x1 buf (4 banks, the per-head U accumulators).
"""

import numpy as np

B = 8
N = 2048
C = 768
H = 4
DH = 192
SCALE = DH ** -0.5

NSC = 2                 # superchunks of 1024 over the sequence
SC = N // NSC           # 1024
HF = SC // 2            # 512 (moving-operand half width)
CC = C // 128           # 6 channel chunks
KT = N // 128           # 16 k-tiles
NWARM = 44

_BUILT = None


def _dest_of(cp):
    h, dd = divmod(cp, DH)
    if dd < 128:
        return ("a", h, dd)
    return ("b", h // 2, (h % 2) * 64 + (dd - 128))


def _jc_segments(jc):
    """Merged PSUM->head-major copy segments for projection j-chunk jc."""
    segs = []
    for p0 in range(0, 128, 64):
        kind, idx, dlo = _dest_of(128 * jc + p0)
        if segs and segs[-1][2] == kind and segs[-1][3] == idx and \
                segs[-1][4] + (segs[-1][1] - segs[-1][0]) == dlo:
            segs[-1] = (segs[-1][0], p0 + 64, kind, idx, segs[-1][4])
        else:
            segs.append((p0, p0 + 64, kind, idx, dlo))
    return segs


def _build():
    from contextlib import ExitStack

    import concourse.mybir as mybir
    import concourse.tile as tile
    from concourse import bacc

    F32 = mybir.dt.float32
    F32R = mybir.dt.float32r
    BF16 = mybir.dt.bfloat16
    AF = mybir.ActivationFunctionType

    nc = bacc.Bacc("TRN2", target_bir_lowering=False, debug=False)
    qt_d = nc.dram_tensor("qT", [C, N], BF16, kind="ExternalInput").ap()
    kt_d = nc.dram_tensor("kT", [C, N], BF16, kind="ExternalInput").ap()
    vt_d = nc.dram_tensor("vT", [C, N], BF16, kind="ExternalInput").ap()
    wqt_d = nc.dram_tensor("WqT", [C, C], BF16, kind="ExternalInput").ap()
    wkt_d = nc.dram_tensor("WkT", [C, C], BF16, kind="ExternalInput").ap()
    wvt_d = nc.dram_tensor("WvT", [C, C], BF16, kind="ExternalInput").ap()
    wpt_d = nc.dram_tensor("WpT", [C, C], BF16, kind="ExternalInput").ap()
    bp_d = nc.dram_tensor("bp", [C], F32, kind="ExternalInput").ap()
    y_d = nc.dram_tensor("y", [N, C], F32, kind="ExternalOutput").ap()

    with tile.TileContext(nc) as tc, ExitStack() as ctx:
        const = ctx.enter_context(tc.tile_pool(name="const", bufs=1))
        wqp = ctx.enter_context(tc.tile_pool(name="wqp", bufs=1))
        khp = ctx.enter_context(tc.tile_pool(name="khp", bufs=1))
        vhp = ctx.enter_context(tc.tile_pool(name="vhp", bufs=1))
        xtp = ctx.enter_context(tc.tile_pool(name="xtp", bufs=2))
        ps = ctx.enter_context(tc.tile_pool(name="ps", bufs=2, space="PSUM"))

        def big_tile(nm):
            return ps.tile([128, SC], F32, tag="big", name=nm, bufs=2)

        ones_col = const.tile([128, H], BF16, tag="ones_col", name="ones_col")
        nc.vector.memset(ones_col[:], 1.0)
        ones_row_f = const.tile([1, 128], F32, tag="ones_row_f",
                                name="ones_row_f")
        nc.vector.memset(ones_row_f[:], 1.0)
        ones_row = const.tile([1, 128], F32R, tag="ones_row", name="ones_row")
        nc.vector.tensor_copy(ones_row[:], ones_row_f[:])

        # PE warm-up: dependency-free matmuls so the HAM clock gate opens
        # while the first DMAs stream in.
        warm_w_f = const.tile([128, 128], F32, tag="warm_w_f", name="warm_w_f")
        nc.vector.memset(warm_w_f[:], 0.5)
        warm_w = const.tile([128, 128], BF16, tag="warm_w", name="warm_w")
        nc.vector.tensor_copy(warm_w[:], warm_w_f[:])
        warm_x = const.tile([128, HF], BF16, tag="warm_x", name="warm_x")
        for i in range(4):
            nc.vector.tensor_copy(warm_x[:, i * 128:(i + 1) * 128], warm_w_f[:])
        for r in range(NWARM):
            wp = ps.tile([128, SC], F32, tag="ua", name="warm_ps", bufs=1)
            nc.tensor.matmul(wp[:, 0:HF], warm_w[:], warm_x[:],
                             start=True, stop=True)

        # ---- persistent weights (direct loads, no transposes) -------------
        WqT = wqp.tile([128, CC, C], BF16, tag="wqt", name="wqt")
        WpT_a = wqp.tile([128, H, C], BF16, tag="wpa", name="wpa")
        WpT_b = [wqp.tile([128, C], BF16, tag=f"wpb{g}", name=f"wpb{g}")
                 for g in range(2)]
        bias_bc = wqp.tile([128, C], F32, tag="bias_bc", name="bias_bc")

        khT_a = [khp.tile([128, N], BF16, tag=f"kha{h}", name=f"kha{h}")
                 for h in range(H)]
        khT_b = [khp.tile([128, N], BF16, tag=f"khb{g}", name=f"khb{g}")
                 for g in range(2)]
        vh = [vhp.tile([128, H, DH + 1], BF16, tag=f"vh{nt}", name=f"vh{nt}")
              for nt in range(KT)]

        def load_wT_grouped(dest, w_dram):
            # dest[p, cc, j] = W.T[cc*128+p, j]
            nc.gpsimd.dma_start(
                dest[:],
                w_dram.rearrange("(cc p) j -> p cc j", p=128))

        def seg_dest(kind, idx, dlo, dhi, a_tiles, b_tiles, col_lo, col_hi):
            t = a_tiles[idx] if kind == "a" else b_tiles[idx]
            return t[dlo:dhi, col_lo:col_hi]

        def in_proj(w_tiles, xTt, a_tiles, b_tiles, n0):
            # out.T[j, n0:n0+SC] head-major packed; one weight load feeds the
            # two 512-col moving halves.
            for jc in range(CC):
                p = big_tile("pj")
                for cc in range(CC):
                    w = w_tiles[:, cc, jc * 128:(jc + 1) * 128]
                    nc.tensor.matmul(p[:, 0:HF], w, xTt[:, cc, 0:HF],
                                     start=(cc == 0), stop=(cc == CC - 1))
                    nc.tensor.matmul(p[:, HF:SC], w, xTt[:, cc, HF:SC],
                                     start=(cc == 0), stop=(cc == CC - 1))
                for (plo, phi, kind, idx, dlo) in _jc_segments(jc):
                    nc.vector.tensor_copy(
                        seg_dest(kind, idx, dlo, dlo + (phi - plo),
                                 a_tiles, b_tiles, n0, n0 + SC),
                        p[plo:phi, :])

        # ---- phase 1: stage k, v ------------------------------------------
        with tc.tile_pool(name="wkv", bufs=1) as wkv:
            WkT = wkv.tile([128, CC, C], BF16, tag="wkt", name="wkt")
            WvT = wkv.tile([128, CC, C], BF16, tag="wvt", name="wvt")
            load_wT_grouped(WkT, wkt_d)

            def load_wq():
                load_wT_grouped(WqT, wqt_d)

            def load_wp_bias():
                # wpt_d is host-packed head-major: rows 0..511 = per-head
                # dd 0..127 (h-major), rows 512..639 / 640..767 = the packed
                # b-tiles (dd 128..191 of heads 0,1 / 2,3).
                nc.gpsimd.dma_start(
                    WpT_a[:],
                    wpt_d[0:512, :].rearrange("(h p) j -> p h j", p=128))
                for g in range(2):
                    nc.gpsimd.dma_start(
                        WpT_b[g][:], wpt_d[512 + g * 128:512 + (g + 1) * 128, :])
                bp_row = wkv.tile([1, C], F32, tag="bp_row", name="bp_row")
                bp_row_r = wkv.tile([1, C], F32R, tag="bp_row_r",
                                    name="bp_row_r")
                nc.sync.dma_start(bp_row[:], bp_d[None, :])
                nc.vector.tensor_copy(bp_row_r[:], bp_row[:])
                pb = big_tile("pb")
                nc.tensor.matmul(pb[:, 0:HF], ones_row[:], bp_row_r[:, 0:HF],
                                 start=True, stop=True)
                nc.tensor.matmul(pb[:, HF:C], ones_row[:], bp_row_r[:, HF:C],
                                 start=True, stop=True)
                nc.scalar.copy(bias_bc[:], pb[:, 0:C])

            for sc in range(NSC):
                n0 = sc * SC
                kTt = xtp.tile([128, CC, SC], BF16, tag="xT", name="kTt")
                nc.gpsimd.dma_start(
                    kTt[:],
                    kt_d[:, n0:n0 + SC].rearrange("(cc p) n -> p cc n", p=128))
                if sc == 0:
                    # WvT queued after the first k staging chunk so the first
                    # k-projection matmuls start ~5us earlier.
                    load_wT_grouped(WvT, wvt_d)
                in_proj(WkT, kTt, khT_a, khT_b, n0)
                vTt = xtp.tile([128, CC, SC], BF16, tag="xT", name="vTt")
                nc.gpsimd.dma_start(
                    vTt[:],
                    vt_d[:, n0:n0 + SC].rearrange("(cc p) n -> p cc n", p=128))
                for ntl in range(SC // 128):
                    nt = sc * (SC // 128) + ntl
                    p = big_tile("pv")
                    for cc in range(CC):
                        xw = vTt[:, cc, ntl * 128:(ntl + 1) * 128]
                        nc.tensor.matmul(p[:, 0:HF], xw, WvT[:, cc, 0:HF],
                                         start=(cc == 0), stop=(cc == CC - 1))
                        nc.tensor.matmul(p[:, HF:C], xw, WvT[:, cc, HF:C],
                                         start=(cc == 0), stop=(cc == CC - 1))
                    nc.vector.tensor_copy(
                        vh[nt][:, :, 0:DH],
                        p[:, 0:C].rearrange("p (h d) -> p h d", h=H))
                    nc.vector.tensor_copy(
                        vh[nt][:, :, DH:DH + 1],
                        ones_col[:].rearrange("p (h o) -> p h o", h=H))
                if sc == 0:
                    load_wq()
                    load_wp_bias()

        # ---- phase 2: per-superchunk attention + output projection --------
        qhp = ctx.enter_context(tc.tile_pool(name="qhp", bufs=1))
        esp = ctx.enter_context(tc.tile_pool(name="esp", bufs=3))
        xop = ctx.enter_context(tc.tile_pool(name="xop", bufs=1))
        scp = ctx.enter_context(tc.tile_pool(name="scp", bufs=2))
        yp = ctx.enter_context(tc.tile_pool(name="yp", bufs=2))
        bcp = ctx.enter_context(tc.tile_pool(name="bcp", bufs=1))

        qhT_a = [qhp.tile([128, SC], BF16, tag=f"qha{h}", name=f"qha{h}")
                 for h in range(H)]
        qhT_b = [qhp.tile([128, SC], BF16, tag=f"qhb{g}", name=f"qhb{g}")
                 for g in range(2)]
        # raw (unnormalized) U, f32 to keep a single bf16 rounding on x
        ur_a = [xop.tile([128, SC], F32, tag=f"ura{h}", name=f"ura{h}")
                for h in range(H)]
        ur_b = [xop.tile([128, SC], F32, tag=f"urb{g}", name=f"urb{g}")
                for g in range(2)]
        rs = [xop.tile([1, SC], F32, tag=f"rs{h}", name=f"rs{h}")
              for h in range(H)]
        xT_a = [xop.tile([128, SC], BF16, tag=f"xta{h}", name=f"xta{h}")
                for h in range(H)]
        xT_b = [xop.tile([128, SC], BF16, tag=f"xtb{g}", name=f"xtb{g}")
                for g in range(2)]

        def q_load(sc):
            n0 = sc * SC
            qTt = xtp.tile([128, CC, SC], BF16, tag="xT", name="qTt")
            nc.gpsimd.dma_start(
                qTt[:],
                qt_d[:, n0:n0 + SC].rearrange("(cc p) n -> p cc n", p=128))
            return qTt

        def finalize_recip(fh, src=None):
            # 1-partition RECIPROCAL of the rowsum row on DVE (iterative
            # divide, ~6.7us at 1024 wide) - slow but fully hidden under the
            # next head's score loop / the output projection.
            recip = scp.tile([1, SC], F32R, tag="recip", name="recip")
            with nc.allow_low_precision(reason="softmax denom recip f32r"):
                nc.vector.reciprocal(recip[:],
                                     rs[fh][:] if src is None else src)
            return recip

        def finalize_muls(fh, recip):
            # broadcast 1/rowsum across partitions with a rank-1 ones matmul,
            # then normalize the raw U tiles into the bf16 stationaries for
            # the output projection.
            g, blo = fh // 2, (fh % 2) * 64
            bc_ps = big_tile("bcp")
            nc.tensor.matmul(bc_ps[:, 0:HF], ones_row[:], recip[:, 0:HF],
                             start=True, stop=True)
            nc.tensor.matmul(bc_ps[:, HF:SC], ones_row[:], recip[:, HF:SC],
                             start=True, stop=True)
            bc = scp.tile([128, SC], F32, tag="bc", name="bc")
            nc.scalar.copy(bc[:], bc_ps[:])
            nc.vector.tensor_mul(xT_a[fh][:], ur_a[fh][:], bc[:])
            nc.vector.tensor_mul(xT_b[g][blo:blo + 64, :],
                                 ur_b[g][blo:blo + 64, :], bc[blo:blo + 64, :])

        def attention(sc):
            # (h, kt) flattened with the score stream leading the av stream
            # by two groups, so the next head's first exps are already in
            # flight when its avs begin - no per-head-boundary PE gap.
            def scores(h, kt):
                g, blo = h // 2, (h % 2) * 64
                s = big_tile("s")
                wa = khT_a[h][:, kt * 128:(kt + 1) * 128]
                qa = qhT_a[h]
                nc.tensor.matmul(s[:, 0:HF], wa, qa[:, 0:HF],
                                 start=True, stop=False)
                nc.tensor.matmul(s[:, HF:SC], wa, qa[:, HF:SC],
                                 start=True, stop=False)
                wb = khT_b[g][blo:blo + 64, kt * 128:(kt + 1) * 128]
                qb = qhT_b[g]
                nc.tensor.matmul(s[:, 0:HF], wb, qb[blo:blo + 64, 0:HF],
                                 start=False, stop=True)
                nc.tensor.matmul(s[:, HF:SC], wb, qb[blo:blo + 64, HF:SC],
                                 start=False, stop=True)
                es = esp.tile([128, SC], BF16, tag="es", name="es")
                nc.scalar.activation(es[:], s[:], AF.Exp, scale=SCALE)
                return es

            def av(h, kt, u_a, u_b, es):
                va = vh[kt][:, h, 0:128]
                st, sp = (kt == 0), (kt == KT - 1)
                nc.tensor.matmul(u_a[:, 0:HF], va, es[:, 0:HF],
                                 start=st, stop=sp)
                nc.tensor.matmul(u_a[:, HF:SC], va, es[:, HF:SC],
                                 start=st, stop=sp)
                vb = vh[kt][:, h, 128:DH + 1]
                nc.tensor.matmul(u_b[:, 0:HF], vb, es[:, 0:HF],
                                 start=st, stop=sp)
                nc.tensor.matmul(u_b[:, HF:SC], vb, es[:, HF:SC],
                                 start=st, stop=sp)

            seq = [(h, kt) for h in range(H) for kt in range(KT)]
            es_q = [scores(*seq[0]), scores(*seq[1])]
            pend = None
            u = None
            for j, (h, kt) in enumerate(seq):
                if j + 2 < len(seq):
                    es_q.append(scores(*seq[j + 2]))
                if kt == 0:
                    u = (ps.tile([128, SC], F32, tag="ua", name="ua", bufs=1),
                         ps.tile([65, SC], F32, tag="ub", name="ub", bufs=1))
                av(h, kt, *u, es_q.pop(0))
                if kt == 2 and pend is not None:
                    pend = (pend[0], finalize_recip(pend[0]))
                elif kt == 8 and pend is not None:
                    finalize_muls(*pend)
                    pend = None
                elif kt == KT - 1:
                    # evacuate raw U immediately so the single-buffered U
                    # psum frees before the next head's first av matmul.
                    # The rowsum row stays in psum for the LAST head: its
                    # reciprocal runs during the projection phase, when no
                    # next av needs the ub buffer, and skipping the staging
                    # copy shortens the normalization chain.
                    g, blo = h // 2, (h % 2) * 64
                    nc.scalar.copy(ur_a[h][:], u[0][:])
                    nc.vector.tensor_copy(ur_b[g][blo:blo + 64, :],
                                          u[1][0:64, :])
                    if h != H - 1:
                        nc.vector.tensor_copy(rs[h][:], u[1][64:65, :])
                    pend = (h,)
            return pend[0], u[1]

        def q_proj_groups(qTt, scalar_evac=True):
            # one projection group per output j-chunk; emitted lazily so the
            # caller can interleave them with the output projection. PSUM
            # evacuation on ScalarE when these groups run while the pending
            # head's 6.5us RECIPROCAL occupies the DVE FIFO (a DVE evac
            # there would stall the PE's psum-buffer rotation behind it);
            # on DVE for the pre-loop instance, where ScalarE copies would
            # instead delay the first attention exps.
            def group(jc):
                p = big_tile("pq")
                for cc in range(CC):
                    w = WqT[:, cc, jc * 128:(jc + 1) * 128]
                    nc.tensor.matmul(p[:, 0:HF], w, qTt[:, cc, 0:HF],
                                     start=(cc == 0), stop=(cc == CC - 1))
                    nc.tensor.matmul(p[:, HF:SC], w, qTt[:, cc, HF:SC],
                                     start=(cc == 0), stop=(cc == CC - 1))
                cp = nc.scalar.copy if scalar_evac else nc.vector.tensor_copy
                for (plo, phi, kind, idx, dlo) in _jc_segments(jc):
                    cp(seg_dest(kind, idx, dlo, dlo + (phi - plo),
                                qhT_a, qhT_b, 0, SC),
                       p[plo:phi, :])
            return [lambda jc=jc: group(jc) for jc in range(CC)]

        def final_part(p, ntl, heads, start, stop):
            # the two heads of a pair share xT_b[g]/WpT_b[g] partition
            # packing, and their b contributions sum into the same output -
            # one full-K=128 matmul replaces two 64-K ones when both heads
            # are present (half the streamed columns).
            ops = [("a", h) for h in heads]
            for g in range(2):
                pair = [h for h in heads if h // 2 == g]
                if len(pair) == 2:
                    ops.append(("b2", g))
                elif pair:
                    ops.append(("b1", pair[0]))
            for i, (kind, v) in enumerate(ops):
                st = start and i == 0
                sp = stop and i == len(ops) - 1
                if kind == "a":
                    w = xT_a[v][:, ntl * 128:(ntl + 1) * 128]
                    ra, rb = WpT_a[:, v, 0:HF], WpT_a[:, v, HF:C]
                elif kind == "b2":
                    w = xT_b[v][:, ntl * 128:(ntl + 1) * 128]
                    ra, rb = WpT_b[v][:, 0:HF], WpT_b[v][:, HF:C]
                else:
                    g, blo = v // 2, (v % 2) * 64
                    w = xT_b[g][blo:blo + 64, ntl * 128:(ntl + 1) * 128]
                    ra = WpT_b[g][blo:blo + 64, 0:HF]
                    rb = WpT_b[g][blo:blo + 64, HF:C]
                nc.tensor.matmul(p[:, 0:HF], w, ra, start=st, stop=sp)
                nc.tensor.matmul(p[:, HF:C], w, rb, start=st, stop=sp)

        def final_evac(p, sc, ntl):
            n0 = sc * SC
            ysb = yp.tile([128, C], F32, tag="y", name="y")
            nc.vector.tensor_add(ysb[:], p[:, 0:C], bias_bc[:])
            nc.sync.dma_start(
                y_d[n0 + ntl * 128:n0 + (ntl + 1) * 128, :], ysb[:])

        def final_ntl(sc, ntl):
            p = big_tile("py")
            final_part(p, ntl, list(range(H)), True, True)
            final_evac(p, sc, ntl)

        def finalize_tail(fh, ub_last):
            # no cover work exists after the last superchunk's attention, so
            # split the pending head's normalization into column quarters -
            # final_ntl(ntl) only needs xT columns [ntl*128,(ntl+1)*128), so
            # the output projection starts after a quarter of the reciprocal,
            # read straight from the rowsum row still sitting in psum.
            # The broadcast psum rides the now-idle "ua" bank pair: the "big"
            # bufs hold the two open part_a groups (deadlock otherwise).
            g, blo = fh // 2, (fh % 2) * 64
            recip = scp.tile([1, SC], F32R, tag="recip", name="recip")
            bc = bcp.tile([128, SC], F32, tag="bcr", name="bcr")
            bc_ps = ps.tile([128, SC], F32, tag="ua", name="bc_ps", bufs=1)
            for pi in range(4):
                lo, hi = pi * (SC // 4), (pi + 1) * (SC // 4)
                with nc.allow_low_precision(reason="softmax denom recip"):
                    nc.vector.reciprocal(recip[:, lo:hi],
                                         ub_last[64:65, lo:hi])
                nc.tensor.matmul(bc_ps[:, lo:hi], ones_row[:],
                                 recip[:, lo:hi], start=True, stop=True)
                nc.scalar.copy(bc[:, lo:hi], bc_ps[:, lo:hi])
                nc.vector.tensor_mul(xT_a[fh][:, lo:hi], ur_a[fh][:, lo:hi],
                                     bc[:, lo:hi])
                nc.vector.tensor_mul(xT_b[g][blo:blo + 64, lo:hi],
                                     ur_b[g][blo:blo + 64, lo:hi],
                                     bc[blo:blo + 64, lo:hi])

        # q-superchunk pipeline: attention(sc) leaves the last head's
        # normalization pending; the next superchunk's q-projection groups
        # are PE work independent of it and cover the reciprocal chain, then
        # the output projection consumes the normalized stationaries.
        qTt = q_load(0)
        for fn in q_proj_groups(qTt, scalar_evac=False):
            fn()
        for sc in range(NSC):
            if sc + 1 < NSC:
                qt_next = q_load(sc + 1)
            fh, ub_last = attention(sc)
            if sc + 1 < NSC:
                recip = finalize_recip(fh, src=ub_last[64:65, :])
                qp = q_proj_groups(qt_next)
                for fn in qp[0:4]:
                    fn()
                finalize_muls(fh, recip)
                rest = qp[4:]
                for ntl in range(SC // 128):
                    final_ntl(sc, ntl)
                    if rest:
                        rest.pop(0)()
            else:
                # tail: open the first two output-projection groups with the
                # already-normalized heads as cover work, weave the pending
                # head's quartered normalization in, then close them.
                heads_a = [h for h in range(H) if h != fh]
                p0 = big_tile("py")
                final_part(p0, 0, heads_a, True, False)
                p1 = big_tile("py")
                final_part(p1, 1, heads_a, True, False)
                finalize_tail(fh, ub_last)
                final_part(p0, 0, [fh], False, True)
                final_evac(p0, sc, 0)
                final_part(p1, 1, [fh], False, True)
                final_evac(p1, sc, 1)
                for ntl in range(2, SC // 128):
                    final_ntl(sc, ntl)

    nc.compile()
    return nc


def _get_built():
    global _BUILT
    if _BUILT is None:
        _BUILT = _build()
    return _BUILT


def run(inputs, trace=False, **kw):
    """Run on all 8 cores; returns (y [B,N,C] float32, BassKernelResults)."""
    import ml_dtypes
    from concourse.bass_utils import run_bass_kernel_spmd

    nc = _get_built()
    bf16 = ml_dtypes.bfloat16
    f32 = np.float32
    wpt = np.asarray(inputs["Wp"], f32).T  # [c', j]
    wpt_packed = np.concatenate(
        [wpt[h * DH:h * DH + 128] for h in range(H)]
        + [wpt[h * DH + 128:(h + 1) * DH] for h in range(H)])
    shared = {
        "WqT": np.ascontiguousarray(np.asarray(inputs["Wq"], f32).T).astype(bf16),
        "WkT": np.ascontiguousarray(np.asarray(inputs["Wk"], f32).T).astype(bf16),
        "WvT": np.ascontiguousarray(np.asarray(inputs["Wv"], f32).T).astype(bf16),
        "WpT": np.ascontiguousarray(wpt_packed).astype(bf16),
        "bp": np.ascontiguousarray(np.asarray(inputs["bp"], f32)),
    }
    q = np.asarray(inputs["q"], f32)
    k = np.asarray(inputs["k"], f32)
    v = np.asarray(inputs["v"], f32)
    in_maps = []
    for b in range(B):
        m = dict(shared)
        m["qT"] = np.ascontiguousarray(q[b].T).astype(bf16)
        m["kT"] = np.ascontiguousarray(k[b].T).astype(bf16)
        m["vT"] = np.ascontiguousarray(v[b].T).astype(bf16)
        in_maps.append(m)
    res = run_bass_kernel_spmd(nc, in_maps, list(range(B)), trace=trace, **kw)
    y = np.stack([res.results[b]["y"] for b in range(B)]).astype(np.float32)
    return y, res


def kernel(q, k, v, Wq, Wk, Wv, Wp, bp):
    y, _ = run({"q": q, "k": k, "v": v, "Wq": Wq, "Wk": Wk, "Wv": Wv,
                "Wp": Wp, "bp": bp})
    return y


# revision 26
# speedup vs baseline: 1.2842x; 1.0009x over previous
"""Trainium2 Bass kernel for nn_Attention_44994077393310.

Multi-head attention (B=8, N=2048, C=768, H=4, Dh=192) with input projections,
softmax attention, and output projection with bias.

Sharding: pure data-parallel over the batch dim - each of the 8 NeuronCores
computes one batch element end-to-end (weights replicated). No collectives.

v2: all matmul operands are bf16 (inputs cast on the host). Rationale from the
v1 (fp32r) trace: the PE was 93% busy but ~1/3 of its time was exposed
LDWEIGHTS - fp32 weights disable the HW fast-weight-load path and every
512-col matmul serialized a ~110ns weight load. bf16 enables FWL, halves DMA
bytes, and the kernel is restructured so consecutive matmul pairs share one
stationary operand (1024-wide q superchunks processed as two 512-col moving
halves per weight load): a repeated weight load hides completely under the
previous matmul, so only genuine stationary CHANGES (~95ns each) remain.
The output projection merges each head pair's 64-row b-part matmuls into one
full-K=128 matmul (their contributions sum in the output and xT_b/WpT_b
already pack the pair on complementary partitions), halving those columns.
fp8/DoubleRow was evaluated and rejected: numpy emulation of e4m3 rounding
puts every injection point (qk, v, es) above the 2e-2 error budget.
Measured HW exec ~435-470us (run-to-run device-state variance is ~+-10%;
identical NEFFs measured 435 and 521us back to back).

Layout strategy (unchanged from v1): q/k/v and weights are pre-transposed on
the host so every DMA lands operands with the contraction dim on partitions.
Scores are computed transposed S_T[key, q] with exp on ScalarE (scale folded
in); a ones column in vh makes softmax denominators fall out of the U = es@v
matmuls; U is evacuated RAW to SBUF (f32) as soon as a head finishes so the
single-buffered U psum frees immediately, and the slow 1-partition RECIPROCAL
+ broadcast + normalize runs on DVE hidden under the next head / the output
projection.

PSUM plan (8 banks): one pool - tag "big" [128,1024]f32 x2 bufs (4 banks,
used by scores/projections/bc broadcast), tags "ua" [128,1024] + "ub"
[65,1024] x1 buf (4 banks, the per-head U accumulators).
"""

import numpy as np

B = 8
N = 2048
C = 768
H = 4
DH = 192
SCALE = DH ** -0.5

NSC = 2                 # superchunks of 1024 over the sequence
SC = N // NSC           # 1024
HF = SC // 2            # 512 (moving-operand half width)
CC = C // 128           # 6 channel chunks
KT = N // 128           # 16 k-tiles
NWARM = 44

_BUILT = None


def _dest_of(cp):
    h, dd = divmod(cp, DH)
    if dd < 128:
        return ("a", h, dd)
    return ("b", h // 2, (h % 2) * 64 + (dd - 128))


def _jc_segments(jc):
    """Merged PSUM->head-major copy segments for projection j-chunk jc."""
    segs = []
    for p0 in range(0, 128, 64):
        kind, idx, dlo = _dest_of(128 * jc + p0)
        if segs and segs[-1][2] == kind and segs[-1][3] == idx and \
                segs[-1][4] + (segs[-1][1] - segs[-1][0]) == dlo:
            segs[-1] = (segs[-1][0], p0 + 64, kind, idx, segs[-1][4])
        else:
            segs.append((p0, p0 + 64, kind, idx, dlo))
    return segs


def _build():
    from contextlib import ExitStack

    import concourse.mybir as mybir
    import concourse.tile as tile
    from concourse import bacc

    F32 = mybir.dt.float32
    F32R = mybir.dt.float32r
    BF16 = mybir.dt.bfloat16
    AF = mybir.ActivationFunctionType

    nc = bacc.Bacc("TRN2", target_bir_lowering=False, debug=False)
    qt_d = nc.dram_tensor("qT", [C, N], BF16, kind="ExternalInput").ap()
    kt_d = nc.dram_tensor("kT", [C, N], BF16, kind="ExternalInput").ap()
    vt_d = nc.dram_tensor("vT", [C, N], BF16, kind="ExternalInput").ap()
    wqt_d = nc.dram_tensor("WqT", [C, C], BF16, kind="ExternalInput").ap()
    wkt_d = nc.dram_tensor("WkT", [C, C], BF16, kind="ExternalInput").ap()
    wvt_d = nc.dram_tensor("WvT", [C, C], BF16, kind="ExternalInput").ap()
    wpt_d = nc.dram_tensor("WpT", [C, C], BF16, kind="ExternalInput").ap()
    bp_d = nc.dram_tensor("bp", [C], F32, kind="ExternalInput").ap()
    y_d = nc.dram_tensor("y", [N, C], F32, kind="ExternalOutput").ap()

    with tile.TileContext(nc) as tc, ExitStack() as ctx:
        const = ctx.enter_context(tc.tile_pool(name="const", bufs=1))
        wqp = ctx.enter_context(tc.tile_pool(name="wqp", bufs=1))
        khp = ctx.enter_context(tc.tile_pool(name="khp", bufs=1))
        vhp = ctx.enter_context(tc.tile_pool(name="vhp", bufs=1))
        xtp = ctx.enter_context(tc.tile_pool(name="xtp", bufs=2))
        ps = ctx.enter_context(tc.tile_pool(name="ps", bufs=2, space="PSUM"))

        def big_tile(nm):
            return ps.tile([128, SC], F32, tag="big", name=nm, bufs=2)

        ones_col = const.tile([128, H], BF16, tag="ones_col", name="ones_col")
        nc.vector.memset(ones_col[:], 1.0)
        ones_row_f = const.tile([1, 128], F32, tag="ones_row_f",
                                name="ones_row_f")
        nc.vector.memset(ones_row_f[:], 1.0)
        ones_row = const.tile([1, 128], F32R, tag="ones_row", name="ones_row")
        nc.vector.tensor_copy(ones_row[:], ones_row_f[:])

        # PE warm-up: dependency-free matmuls so the HAM clock gate opens
        # while the first DMAs stream in.
        warm_w_f = const.tile([128, 128], F32, tag="warm_w_f", name="warm_w_f")
        nc.vector.memset(warm_w_f[:], 0.5)
        warm_w = const.tile([128, 128], BF16, tag="warm_w", name="warm_w")
        nc.vector.tensor_copy(warm_w[:], warm_w_f[:])
        warm_x = const.tile([128, HF], BF16, tag="warm_x", name="warm_x")
        for i in range(4):
            nc.vector.tensor_copy(warm_x[:, i * 128:(i + 1) * 128], warm_w_f[:])
        for r in range(NWARM):
            wp = ps.tile([128, SC], F32, tag="ua", name="warm_ps", bufs=1)
            nc.tensor.matmul(wp[:, 0:HF], warm_w[:], warm_x[:],
                             start=True, stop=True)

        # ---- persistent weights (direct loads, no transposes) -------------
        WqT = wqp.tile([128, CC, C], BF16, tag="wqt", name="wqt")
        WpT_a = wqp.tile([128, H, C], BF16, tag="wpa", name="wpa")
        WpT_b = [wqp.tile([128, C], BF16, tag=f"wpb{g}", name=f"wpb{g}")
                 for g in range(2)]
        bias_bc = wqp.tile([128, C], F32, tag="bias_bc", name="bias_bc")

        khT_a = [khp.tile([128, N], BF16, tag=f"kha{h}", name=f"kha{h}")
                 for h in range(H)]
        # per-head b-dim tiles with the 64 rows DUPLICATED on both partition
        # halves: the two 512-col score matmuls of a head then run as 64-row
        # PE tiles at positions (0,0)/(64,0) - disjoint row groups and psum
        # banks, so the hardware overlaps them (and pulls the second
        # LDWEIGHTS ahead), halving the b-part's streamed time.
        khT_b = [khp.tile([128, N], BF16, tag=f"khb{h}", name=f"khb{h}")
                 for h in range(H)]
        vh = [vhp.tile([128, H, DH + 1], BF16, tag=f"vh{nt}", name=f"vh{nt}")
              for nt in range(KT)]

        def load_wT_grouped(dest, w_dram):
            # dest[p, cc, j] = W.T[cc*128+p, j]
            nc.gpsimd.dma_start(
                dest[:],
                w_dram.rearrange("(cc p) j -> p cc j", p=128))

        def evac_jc(p, jc, a_tiles, b_tiles, col_lo, col_hi, cp):
            for (plo, phi, kind, idx, dlo) in _jc_segments(jc):
                if kind == "a":
                    cp(a_tiles[idx][dlo:dlo + (phi - plo), col_lo:col_hi],
                       p[plo:phi, :])
                else:
                    for off in range(0, phi - plo, 64):
                        h = 2 * idx + (1 if dlo + off >= 64 else 0)
                        s64 = p[plo + off:plo + off + 64, :]
                        cp(b_tiles[h][0:64, col_lo:col_hi], s64)
                        cp(b_tiles[h][64:128, col_lo:col_hi], s64)

        def in_proj(w_tiles, xTt, a_tiles, b_tiles, n0):
            # out.T[j, n0:n0+SC] head-major packed; one weight load feeds the
            # two 512-col moving halves.
            for jc in range(CC):
                p = big_tile("pj")
                for cc in range(CC):
                    w = w_tiles[:, cc, jc * 128:(jc + 1) * 128]
                    nc.tensor.matmul(p[:, 0:HF], w, xTt[:, cc, 0:HF],
                                     start=(cc == 0), stop=(cc == CC - 1))
                    nc.tensor.matmul(p[:, HF:SC], w, xTt[:, cc, HF:SC],
                                     start=(cc == 0), stop=(cc == CC - 1))
                evac_jc(p, jc, a_tiles, b_tiles, n0, n0 + SC,
                        nc.vector.tensor_copy)

        # ---- phase 1: stage k, v ------------------------------------------
        with tc.tile_pool(name="wkv", bufs=1) as wkv:
            WkT = wkv.tile([128, CC, C], BF16, tag="wkt", name="wkt")
            WvT = wkv.tile([128, CC, C], BF16, tag="wvt", name="wvt")
            load_wT_grouped(WkT, wkt_d)

            def load_wq():
                load_wT_grouped(WqT, wqt_d)

            def load_wp_bias():
                # wpt_d is host-packed head-major: rows 0..511 = per-head
                # dd 0..127 (h-major), rows 512..639 / 640..767 = the packed
                # b-tiles (dd 128..191 of heads 0,1 / 2,3).
                nc.gpsimd.dma_start(
                    WpT_a[:],
                    wpt_d[0:512, :].rearrange("(h p) j -> p h j", p=128))
                for g in range(2):
                    nc.gpsimd.dma_start(
                        WpT_b[g][:], wpt_d[512 + g * 128:512 + (g + 1) * 128, :])
                bp_row = wkv.tile([1, C], F32, tag="bp_row", name="bp_row")
                bp_row_r = wkv.tile([1, C], F32R, tag="bp_row_r",
                                    name="bp_row_r")
                nc.sync.dma_start(bp_row[:], bp_d[None, :])
                nc.vector.tensor_copy(bp_row_r[:], bp_row[:])
                pb = big_tile("pb")
                nc.tensor.matmul(pb[:, 0:HF], ones_row[:], bp_row_r[:, 0:HF],
                                 start=True, stop=True)
                nc.tensor.matmul(pb[:, HF:C], ones_row[:], bp_row_r[:, HF:C],
                                 start=True, stop=True)
                nc.scalar.copy(bias_bc[:], pb[:, 0:C])

            for sc in range(NSC):
                n0 = sc * SC
                kTt = xtp.tile([128, CC, SC], BF16, tag="xT", name="kTt")
                nc.gpsimd.dma_start(
                    kTt[:],
                    kt_d[:, n0:n0 + SC].rearrange("(cc p) n -> p cc n", p=128))
                if sc == 0:
                    # WvT queued after the first k staging chunk so the first
                    # k-projection matmuls start ~5us earlier.
                    load_wT_grouped(WvT, wvt_d)
                in_proj(WkT, kTt, khT_a, khT_b, n0)
                vTt = xtp.tile([128, CC, SC], BF16, tag="xT", name="vTt")
                nc.gpsimd.dma_start(
                    vTt[:],
                    vt_d[:, n0:n0 + SC].rearrange("(cc p) n -> p cc n", p=128))
                for ntl in range(SC // 128):
                    nt = sc * (SC // 128) + ntl
                    p = big_tile("pv")
                    for cc in range(CC):
                        xw = vTt[:, cc, ntl * 128:(ntl + 1) * 128]
                        nc.tensor.matmul(p[:, 0:HF], xw, WvT[:, cc, 0:HF],
                                         start=(cc == 0), stop=(cc == CC - 1))
                        nc.tensor.matmul(p[:, HF:C], xw, WvT[:, cc, HF:C],
                                         start=(cc == 0), stop=(cc == CC - 1))
                    nc.vector.tensor_copy(
                        vh[nt][:, :, 0:DH],
                        p[:, 0:C].rearrange("p (h d) -> p h d", h=H))
                    nc.vector.tensor_copy(
                        vh[nt][:, :, DH:DH + 1],
                        ones_col[:].rearrange("p (h o) -> p h o", h=H))
                if sc == 0:
                    load_wq()
                    load_wp_bias()

        # ---- phase 2: per-superchunk attention + output projection --------
        qhp = ctx.enter_context(tc.tile_pool(name="qhp", bufs=1))
        esp = ctx.enter_context(tc.tile_pool(name="esp", bufs=3))
        xop = ctx.enter_context(tc.tile_pool(name="xop", bufs=1))
        scp = ctx.enter_context(tc.tile_pool(name="scp", bufs=2))
        yp = ctx.enter_context(tc.tile_pool(name="yp", bufs=2))
        bcp = ctx.enter_context(tc.tile_pool(name="bcp", bufs=1))

        qhT_a = [qhp.tile([128, SC], BF16, tag=f"qha{h}", name=f"qha{h}")
                 for h in range(H)]
        qhT_b = [qhp.tile([128, SC], BF16, tag=f"qhb{h}", name=f"qhb{h}")
                 for h in range(H)]
        # raw (unnormalized) U, f32 to keep a single bf16 rounding on x
        ur_a = [xop.tile([128, SC], F32, tag=f"ura{h}", name=f"ura{h}")
                for h in range(H)]
        ur_b = [xop.tile([128, SC], F32, tag=f"urb{g}", name=f"urb{g}")
                for g in range(2)]
        rs = [xop.tile([1, SC], F32, tag=f"rs{h}", name=f"rs{h}")
              for h in range(H)]
        xT_a = [xop.tile([128, SC], BF16, tag=f"xta{h}", name=f"xta{h}")
                for h in range(H)]
        xT_b = [xop.tile([128, SC], BF16, tag=f"xtb{g}", name=f"xtb{g}")
                for g in range(2)]

        def q_load(sc):
            n0 = sc * SC
            qTt = xtp.tile([128, CC, SC], BF16, tag="xT", name="qTt")
            nc.gpsimd.dma_start(
                qTt[:],
                qt_d[:, n0:n0 + SC].rearrange("(cc p) n -> p cc n", p=128))
            return qTt

        def finalize_recip(fh, src=None):
            # 1-partition RECIPROCAL of the rowsum row on DVE (iterative
            # divide, ~6.7us at 1024 wide) - slow but fully hidden under the
            # next head's score loop / the output projection.
            recip = scp.tile([1, SC], F32R, tag="recip", name="recip")
            with nc.allow_low_precision(reason="softmax denom recip f32r"):
                nc.vector.reciprocal(recip[:],
                                     rs[fh][:] if src is None else src)
            return recip

        def finalize_muls(fh, recip):
            # broadcast 1/rowsum across partitions with a rank-1 ones matmul,
            # then normalize the raw U tiles into the bf16 stationaries for
            # the output projection.
            g, blo = fh // 2, (fh % 2) * 64
            bc_ps = big_tile("bcp")
            nc.tensor.matmul(bc_ps[:, 0:HF], ones_row[:], recip[:, 0:HF],
                             start=True, stop=True)
            nc.tensor.matmul(bc_ps[:, HF:SC], ones_row[:], recip[:, HF:SC],
                             start=True, stop=True)
            bc = scp.tile([128, SC], F32, tag="bc", name="bc")
            nc.scalar.copy(bc[:], bc_ps[:])
            nc.vector.tensor_mul(xT_a[fh][:], ur_a[fh][:], bc[:])
            nc.vector.tensor_mul(xT_b[g][blo:blo + 64, :],
                                 ur_b[g][blo:blo + 64, :], bc[blo:blo + 64, :])

        def attention(sc):
            # (h, kt) flattened with the score stream leading the av stream
            # by two groups, so the next head's first exps are already in
            # flight when its avs begin - no per-head-boundary PE gap.
            def scores(h, kt):
                g, blo = h // 2, (h % 2) * 64
                s = big_tile("s")
                wa = khT_a[h][:, kt * 128:(kt + 1) * 128]
                qa = qhT_a[h]
                nc.tensor.matmul(s[:, 0:HF], wa, qa[:, 0:HF],
                                 start=True, stop=False)
                nc.tensor.matmul(s[:, HF:SC], wa, qa[:, HF:SC],
                                 start=True, stop=False)
                wb, qb = khT_b[h], qhT_b[h]
                nc.tensor.matmul(s[:, 0:HF],
                                 wb[0:64, kt * 128:(kt + 1) * 128],
                                 qb[0:64, 0:HF], start=False, stop=True)
                nc.tensor.matmul(s[:, HF:SC],
                                 wb[64:128, kt * 128:(kt + 1) * 128],
                                 qb[64:128, HF:SC], start=False, stop=True)
                es = esp.tile([128, SC], BF16, tag="es", name="es")
                nc.scalar.activation(es[:], s[:], AF.Exp, scale=SCALE)
                return es

            def av(h, kt, u_a, u_b, es):
                va = vh[kt][:, h, 0:128]
                st, sp = (kt == 0), (kt == KT - 1)
                nc.tensor.matmul(u_a[:, 0:HF], va, es[:, 0:HF],
                                 start=st, stop=sp)
                nc.tensor.matmul(u_a[:, HF:SC], va, es[:, HF:SC],
                                 start=st, stop=sp)
                vb = vh[kt][:, h, 128:DH + 1]
                nc.tensor.matmul(u_b[:, 0:HF], vb, es[:, 0:HF],
                                 start=st, stop=sp)
                nc.tensor.matmul(u_b[:, HF:SC], vb, es[:, HF:SC],
                                 start=st, stop=sp)

            seq = [(h, kt) for h in range(H) for kt in range(KT)]
            es_q = [scores(*seq[0]), scores(*seq[1])]
            pend = None
            u = None
            for j, (h, kt) in enumerate(seq):
                if j + 2 < len(seq):
                    es_q.append(scores(*seq[j + 2]))
                if kt == 0:
                    u = (ps.tile([128, SC], F32, tag="ua", name="ua", bufs=1),
                         ps.tile([65, SC], F32, tag="ub", name="ub", bufs=1))
                av(h, kt, *u, es_q.pop(0))
                if kt == 2 and pend is not None:
                    pend = (pend[0], finalize_recip(pend[0]))
                elif kt == 8 and pend is not None:
                    finalize_muls(*pend)
                    pend = None
                elif kt == KT - 1:
                    # evacuate raw U immediately so the single-buffered U
                    # psum frees before the next head's first av matmul.
                    # The rowsum row stays in psum for the LAST head: its
                    # reciprocal runs during the projection phase, when no
                    # next av needs the ub buffer, and skipping the staging
                    # copy shortens the normalization chain.
                    g, blo = h // 2, (h % 2) * 64
                    nc.scalar.copy(ur_a[h][:], u[0][:])
                    nc.vector.tensor_copy(ur_b[g][blo:blo + 64, :],
                                          u[1][0:64, :])
                    if h != H - 1:
                        nc.vector.tensor_copy(rs[h][:], u[1][64:65, :])
                    pend = (h,)
            return pend[0], u[1]

        def q_proj_groups(qTt, scalar_evac=True):
            # one projection group per output j-chunk; emitted lazily so the
            # caller can interleave them with the output projection. PSUM
            # evacuation on ScalarE when these groups run while the pending
            # head's 6.5us RECIPROCAL occupies the DVE FIFO (a DVE evac
            # there would stall the PE's psum-buffer rotation behind it);
            # on DVE for the pre-loop instance, where ScalarE copies would
            # instead delay the first attention exps.
            def group(jc):
                p = big_tile("pq")
                for cc in range(CC):
                    w = WqT[:, cc, jc * 128:(jc + 1) * 128]
                    nc.tensor.matmul(p[:, 0:HF], w, qTt[:, cc, 0:HF],
                                     start=(cc == 0), stop=(cc == CC - 1))
                    nc.tensor.matmul(p[:, HF:SC], w, qTt[:, cc, HF:SC],
                                     start=(cc == 0), stop=(cc == CC - 1))
                cp = nc.scalar.copy if scalar_evac else nc.vector.tensor_copy
                evac_jc(p, jc, qhT_a, qhT_b, 0, SC, cp)
            return [lambda jc=jc: group(jc) for jc in range(CC)]

        def final_part(p, ntl, heads, start, stop):
            # the two heads of a pair share xT_b[g]/WpT_b[g] partition
            # packing, and their b contributions sum into the same output -
            # one full-K=128 matmul replaces two 64-K ones when both heads
            # are present (half the streamed columns).
            ops = [("a", h) for h in heads]
            for g in range(2):
                pair = [h for h in heads if h // 2 == g]
                if len(pair) == 2:
                    ops.append(("b2", g))
                elif pair:
                    ops.append(("b1", pair[0]))
            for i, (kind, v) in enumerate(ops):
                st = start and i == 0
                sp = stop and i == len(ops) - 1
                if kind == "a":
                    w = xT_a[v][:, ntl * 128:(ntl + 1) * 128]
                    ra, rb = WpT_a[:, v, 0:HF], WpT_a[:, v, HF:C]
                elif kind == "b2":
                    w = xT_b[v][:, ntl * 128:(ntl + 1) * 128]
                    ra, rb = WpT_b[v][:, 0:HF], WpT_b[v][:, HF:C]
                else:
                    g, blo = v // 2, (v % 2) * 64
                    w = xT_b[g][blo:blo + 64, ntl * 128:(ntl + 1) * 128]
                    ra = WpT_b[g][blo:blo + 64, 0:HF]
                    rb = WpT_b[g][blo:blo + 64, HF:C]
                nc.tensor.matmul(p[:, 0:HF], w, ra, start=st, stop=sp)
                nc.tensor.matmul(p[:, HF:C], w, rb, start=st, stop=sp)

        def final_evac(p, sc, ntl):
            n0 = sc * SC
            ysb = yp.tile([128, C], F32, tag="y", name="y")
            nc.vector.tensor_add(ysb[:], p[:, 0:C], bias_bc[:])
            nc.sync.dma_start(
                y_d[n0 + ntl * 128:n0 + (ntl + 1) * 128, :], ysb[:])

        def final_ntl(sc, ntl):
            p = big_tile("py")
            final_part(p, ntl, list(range(H)), True, True)
            final_evac(p, sc, ntl)

        def finalize_tail(fh, ub_last):
            # no cover work exists after the last superchunk's attention, so
            # split the pending head's normalization into column quarters -
            # final_ntl(ntl) only needs xT columns [ntl*128,(ntl+1)*128), so
            # the output projection starts after a quarter of the reciprocal,
            # read straight from the rowsum row still sitting in psum.
            # The broadcast psum rides the now-idle "ua" bank pair: the "big"
            # bufs hold the two open part_a groups (deadlock otherwise).
            g, blo = fh // 2, (fh % 2) * 64
            recip = scp.tile([1, SC], F32R, tag="recip", name="recip")
            bc = bcp.tile([128, SC], F32, tag="bcr", name="bcr")
            bc_ps = ps.tile([128, SC], F32, tag="ua", name="bc_ps", bufs=1)
            for pi in range(4):
                lo, hi = pi * (SC // 4), (pi + 1) * (SC // 4)
                with nc.allow_low_precision(reason="softmax denom recip"):
                    nc.vector.reciprocal(recip[:, lo:hi],
                                         ub_last[64:65, lo:hi])
                nc.tensor.matmul(bc_ps[:, lo:hi], ones_row[:],
                                 recip[:, lo:hi], start=True, stop=True)
                nc.scalar.copy(bc[:, lo:hi], bc_ps[:, lo:hi])
                nc.vector.tensor_mul(xT_a[fh][:, lo:hi], ur_a[fh][:, lo:hi],
                                     bc[:, lo:hi])
                nc.vector.tensor_mul(xT_b[g][blo:blo + 64, lo:hi],
                                     ur_b[g][blo:blo + 64, lo:hi],
                                     bc[blo:blo + 64, lo:hi])

        # q-superchunk pipeline: attention(sc) leaves the last head's
        # normalization pending; the next superchunk's q-projection groups
        # are PE work independent of it and cover the reciprocal chain, then
        # the output projection consumes the normalized stationaries.
        qTt = q_load(0)
        for fn in q_proj_groups(qTt, scalar_evac=False):
            fn()
        for sc in range(NSC):
            if sc + 1 < NSC:
                qt_next = q_load(sc + 1)
            fh, ub_last = attention(sc)
            if sc + 1 < NSC:
                recip = finalize_recip(fh, src=ub_last[64:65, :])
                qp = q_proj_groups(qt_next)
                for fn in qp[0:4]:
                    fn()
                finalize_muls(fh, recip)
                rest = qp[4:]
                for ntl in range(SC // 128):
                    final_ntl(sc, ntl)
                    if rest:
                        rest.pop(0)()
            else:
                # tail: open the first two output-projection groups with the
                # already-normalized heads as cover work, weave the pending
                # head's quartered normalization in, then close them.
                heads_a = [h for h in range(H) if h != fh]
                p0 = big_tile("py")
                final_part(p0, 0, heads_a, True, False)
                p1 = big_tile("py")
                final_part(p1, 1, heads_a, True, False)
                finalize_tail(fh, ub_last)
                final_part(p0, 0, [fh], False, True)
                final_evac(p0, sc, 0)
                final_part(p1, 1, [fh], False, True)
                final_evac(p1, sc, 1)
                for ntl in range(2, SC // 128):
                    final_ntl(sc, ntl)

    nc.compile()
    return nc


def _get_built():
    global _BUILT
    if _BUILT is None:
        _BUILT = _build()
    return _BUILT


def run(inputs, trace=False, **kw):
    """Run on all 8 cores; returns (y [B,N,C] float32, BassKernelResults)."""
    import ml_dtypes
    from concourse.bass_utils import run_bass_kernel_spmd

    nc = _get_built()
    bf16 = ml_dtypes.bfloat16
    f32 = np.float32
    wpt = np.asarray(inputs["Wp"], f32).T  # [c', j]
    wpt_packed = np.concatenate(
        [wpt[h * DH:h * DH + 128] for h in range(H)]
        + [wpt[h * DH + 128:(h + 1) * DH] for h in range(H)])
    shared = {
        "WqT": np.ascontiguousarray(np.asarray(inputs["Wq"], f32).T).astype(bf16),
        "WkT": np.ascontiguousarray(np.asarray(inputs["Wk"], f32).T).astype(bf16),
        "WvT": np.ascontiguousarray(np.asarray(inputs["Wv"], f32).T).astype(bf16),
        "WpT": np.ascontiguousarray(wpt_packed).astype(bf16),
        "bp": np.ascontiguousarray(np.asarray(inputs["bp"], f32)),
    }
    q = np.asarray(inputs["q"], f32)
    k = np.asarray(inputs["k"], f32)
    v = np.asarray(inputs["v"], f32)
    in_maps = []
    for b in range(B):
        m = dict(shared)
        m["qT"] = np.ascontiguousarray(q[b].T).astype(bf16)
        m["kT"] = np.ascontiguousarray(k[b].T).astype(bf16)
        m["vT"] = np.ascontiguousarray(v[b].T).astype(bf16)
        in_maps.append(m)
    res = run_bass_kernel_spmd(nc, in_maps, list(range(B)), trace=trace, **kw)
    y = np.stack([res.results[b]["y"] for b in range(B)]).astype(np.float32)
    return y, res


def kernel(q, k, v, Wq, Wk, Wv, Wp, bp):
    y, _ = run({"q": q, "k": k, "v": v, "Wq": Wq, "Wk": Wk, "Wv": Wv,
                "Wp": Wp, "bp": bp})
    return y


# revision 28
# speedup vs baseline: 1.3070x; 1.0177x over previous
"""Trainium2 Bass kernel for nn_Attention_44994077393310.

Multi-head attention (B=8, N=2048, C=768, H=4, Dh=192) with input projections,
softmax attention, and output projection with bias.

Sharding: pure data-parallel over the batch dim - each of the 8 NeuronCores
computes one batch element end-to-end (weights replicated). No collectives.

v2: all matmul operands are bf16 (inputs cast on the host). Rationale from the
v1 (fp32r) trace: the PE was 93% busy but ~1/3 of its time was exposed
LDWEIGHTS - fp32 weights disable the HW fast-weight-load path and every
512-col matmul serialized a ~110ns weight load. bf16 enables FWL, halves DMA
bytes, and the kernel is restructured so consecutive matmul pairs share one
stationary operand (1024-wide q superchunks processed as two 512-col moving
halves per weight load): a repeated weight load hides completely under the
previous matmul, so only genuine stationary CHANGES (~95ns each) remain.
The output projection merges each head pair's 64-row b-part matmuls into one
full-K=128 matmul (their contributions sum in the output and xT_b/WpT_b
already pack the pair on complementary partitions), halving those columns.
fp8/DoubleRow was evaluated and rejected: numpy emulation of e4m3 rounding
puts every injection point (qk, v, es) above the 2e-2 error budget.
Measured HW exec ~435-470us (run-to-run device-state variance is ~+-10%;
identical NEFFs measured 435 and 521us back to back).

Layout strategy (unchanged from v1): q/k/v and weights are pre-transposed on
the host so every DMA lands operands with the contraction dim on partitions.
Scores are computed transposed S_T[key, q] with exp on ScalarE (scale folded
in); a ones column in vh makes softmax denominators fall out of the U = es@v
matmuls; U is evacuated RAW to SBUF (f32) as soon as a head finishes so the
single-buffered U psum frees immediately, and the slow 1-partition RECIPROCAL
+ broadcast + normalize runs on DVE hidden under the next head / the output
projection.

PSUM plan (8 banks): one pool - tag "big" [128,1024]f32 x2 bufs (4 banks,
used by scores/projections/bc broadcast), tags "ua" [128,1024] + "ub"
[65,1024] x1 buf (4 banks, the per-head U accumulators).
"""

import numpy as np

B = 8
N = 2048
C = 768
H = 4
DH = 192
SCALE = DH ** -0.5

NSC = 2                 # superchunks of 1024 over the sequence
SC = N // NSC           # 1024
HF = SC // 2            # 512 (moving-operand half width)
CC = C // 128           # 6 channel chunks
KT = N // 128           # 16 k-tiles
NWARM = 44

_BUILT = None


def _dest_of(cp):
    h, dd = divmod(cp, DH)
    if dd < 128:
        return ("a", h, dd)
    return ("b", h // 2, (h % 2) * 64 + (dd - 128))


def _jc_segments(jc):
    """Merged PSUM->head-major copy segments for projection j-chunk jc."""
    segs = []
    for p0 in range(0, 128, 64):
        kind, idx, dlo = _dest_of(128 * jc + p0)
        if segs and segs[-1][2] == kind and segs[-1][3] == idx and \
                segs[-1][4] + (segs[-1][1] - segs[-1][0]) == dlo:
            segs[-1] = (segs[-1][0], p0 + 64, kind, idx, segs[-1][4])
        else:
            segs.append((p0, p0 + 64, kind, idx, dlo))
    return segs


def _build():
    from contextlib import ExitStack

    import concourse.mybir as mybir
    import concourse.tile as tile
    from concourse import bacc

    F32 = mybir.dt.float32
    F32R = mybir.dt.float32r
    BF16 = mybir.dt.bfloat16
    AF = mybir.ActivationFunctionType

    nc = bacc.Bacc("TRN2", target_bir_lowering=False, debug=False)
    qt_d = nc.dram_tensor("qT", [C, N], BF16, kind="ExternalInput").ap()
    kt_d = nc.dram_tensor("kT", [C, N], BF16, kind="ExternalInput").ap()
    vt_d = nc.dram_tensor("vT", [C, N], BF16, kind="ExternalInput").ap()
    wqt_d = nc.dram_tensor("WqT", [C, C], BF16, kind="ExternalInput").ap()
    wkt_d = nc.dram_tensor("WkT", [C, C], BF16, kind="ExternalInput").ap()
    wvt_d = nc.dram_tensor("WvT", [C, C], BF16, kind="ExternalInput").ap()
    wpt_d = nc.dram_tensor("WpT", [C, C], BF16, kind="ExternalInput").ap()
    bp_d = nc.dram_tensor("bp", [C], F32, kind="ExternalInput").ap()
    y_d = nc.dram_tensor("y", [N, C], F32, kind="ExternalOutput").ap()

    with tile.TileContext(nc) as tc, ExitStack() as ctx:
        const = ctx.enter_context(tc.tile_pool(name="const", bufs=1))
        wqp = ctx.enter_context(tc.tile_pool(name="wqp", bufs=1))
        khp = ctx.enter_context(tc.tile_pool(name="khp", bufs=1))
        vhp = ctx.enter_context(tc.tile_pool(name="vhp", bufs=1))
        xtp = ctx.enter_context(tc.tile_pool(name="xtp", bufs=2))
        ps = ctx.enter_context(tc.tile_pool(name="ps", bufs=2, space="PSUM"))

        def big_tile(nm):
            return ps.tile([128, SC], F32, tag="big", name=nm, bufs=2)

        ones_col = const.tile([128, H], BF16, tag="ones_col", name="ones_col")
        nc.vector.memset(ones_col[:], 1.0)
        ones_row_f = const.tile([1, 128], F32, tag="ones_row_f",
                                name="ones_row_f")
        nc.vector.memset(ones_row_f[:], 1.0)
        ones_row = const.tile([1, 128], F32R, tag="ones_row", name="ones_row")
        nc.vector.tensor_copy(ones_row[:], ones_row_f[:])

        # PE warm-up: dependency-free matmuls so the HAM clock gate opens
        # while the first DMAs stream in.
        warm_w_f = const.tile([128, 128], F32, tag="warm_w_f", name="warm_w_f")
        nc.vector.memset(warm_w_f[:], 0.5)
        warm_w = const.tile([128, 128], BF16, tag="warm_w", name="warm_w")
        nc.vector.tensor_copy(warm_w[:], warm_w_f[:])
        warm_x = const.tile([128, HF], BF16, tag="warm_x", name="warm_x")
        for i in range(4):
            nc.vector.tensor_copy(warm_x[:, i * 128:(i + 1) * 128], warm_w_f[:])
        for r in range(NWARM):
            wp = ps.tile([128, SC], F32, tag="ua", name="warm_ps", bufs=1)
            nc.tensor.matmul(wp[:, 0:HF], warm_w[:], warm_x[:],
                             start=True, stop=True)

        # ---- persistent weights (direct loads, no transposes) -------------
        WqT = wqp.tile([128, CC, C], BF16, tag="wqt", name="wqt")
        WpT_a = wqp.tile([128, H, C], BF16, tag="wpa", name="wpa")
        WpT_b = [wqp.tile([128, C], BF16, tag=f"wpb{g}", name=f"wpb{g}")
                 for g in range(2)]
        bias_bc = wqp.tile([128, C], F32, tag="bias_bc", name="bias_bc")

        khT_a = [khp.tile([128, N], BF16, tag=f"kha{h}", name=f"kha{h}")
                 for h in range(H)]
        # per-head b-dim tiles with the 64 rows DUPLICATED on both partition
        # halves: the two 512-col score matmuls of a head then run as 64-row
        # PE tiles at positions (0,0)/(64,0) - disjoint row groups and psum
        # banks, so the hardware overlaps them (and pulls the second
        # LDWEIGHTS ahead), halving the b-part's streamed time.
        khT_b = [khp.tile([128, N], BF16, tag=f"khb{h}", name=f"khb{h}")
                 for h in range(H)]
        vh = [vhp.tile([128, H, DH + 1], BF16, tag=f"vh{nt}", name=f"vh{nt}")
              for nt in range(KT)]

        def load_wT_grouped(dest, w_dram):
            # dest[p, cc, j] = W.T[cc*128+p, j]
            nc.gpsimd.dma_start(
                dest[:],
                w_dram.rearrange("(cc p) j -> p cc j", p=128))

        def evac_jc(p, jc, a_tiles, b_tiles, col_lo, col_hi, cp, cp2=None):
            # cp2 (a second engine) takes the duplicate b-half copy so the
            # doubled evacuation never becomes the projection bottleneck.
            cp2 = cp2 or cp
            for (plo, phi, kind, idx, dlo) in _jc_segments(jc):
                if kind == "a":
                    cp(a_tiles[idx][dlo:dlo + (phi - plo), col_lo:col_hi],
                       p[plo:phi, :])
                else:
                    for off in range(0, phi - plo, 64):
                        h = 2 * idx + (1 if dlo + off >= 64 else 0)
                        s64 = p[plo + off:plo + off + 64, :]
                        cp(b_tiles[h][0:64, col_lo:col_hi], s64)
                        cp2(b_tiles[h][64:128, col_lo:col_hi], s64)

        def in_proj(w_tiles, xTt, a_tiles, b_tiles, n0):
            # out.T[j, n0:n0+SC] head-major packed; one weight load feeds the
            # two 512-col moving halves.
            for jc in range(CC):
                p = big_tile("pj")
                for cc in range(CC):
                    w = w_tiles[:, cc, jc * 128:(jc + 1) * 128]
                    nc.tensor.matmul(p[:, 0:HF], w, xTt[:, cc, 0:HF],
                                     start=(cc == 0), stop=(cc == CC - 1))
                    nc.tensor.matmul(p[:, HF:SC], w, xTt[:, cc, HF:SC],
                                     start=(cc == 0), stop=(cc == CC - 1))
                evac_jc(p, jc, a_tiles, b_tiles, n0, n0 + SC,
                        nc.vector.tensor_copy, nc.scalar.copy)

        # ---- phase 1: stage k, v ------------------------------------------
        with tc.tile_pool(name="wkv", bufs=1) as wkv:
            WkT = wkv.tile([128, CC, C], BF16, tag="wkt", name="wkt")
            WvT = wkv.tile([128, CC, C], BF16, tag="wvt", name="wvt")
            load_wT_grouped(WkT, wkt_d)

            def load_wq():
                load_wT_grouped(WqT, wqt_d)

            def load_wp_bias():
                # wpt_d is host-packed head-major: rows 0..511 = per-head
                # dd 0..127 (h-major), rows 512..639 / 640..767 = the packed
                # b-tiles (dd 128..191 of heads 0,1 / 2,3).
                nc.gpsimd.dma_start(
                    WpT_a[:],
                    wpt_d[0:512, :].rearrange("(h p) j -> p h j", p=128))
                for g in range(2):
                    nc.gpsimd.dma_start(
                        WpT_b[g][:], wpt_d[512 + g * 128:512 + (g + 1) * 128, :])
                bp_row = wkv.tile([1, C], F32, tag="bp_row", name="bp_row")
                bp_row_r = wkv.tile([1, C], F32R, tag="bp_row_r",
                                    name="bp_row_r")
                nc.sync.dma_start(bp_row[:], bp_d[None, :])
                nc.vector.tensor_copy(bp_row_r[:], bp_row[:])
                pb = big_tile("pb")
                nc.tensor.matmul(pb[:, 0:HF], ones_row[:], bp_row_r[:, 0:HF],
                                 start=True, stop=True)
                nc.tensor.matmul(pb[:, HF:C], ones_row[:], bp_row_r[:, HF:C],
                                 start=True, stop=True)
                nc.scalar.copy(bias_bc[:], pb[:, 0:C])

            for sc in range(NSC):
                n0 = sc * SC
                kTt = xtp.tile([128, CC, SC], BF16, tag="xT", name="kTt")
                nc.gpsimd.dma_start(
                    kTt[:],
                    kt_d[:, n0:n0 + SC].rearrange("(cc p) n -> p cc n", p=128))
                if sc == 0:
                    # WvT queued after the first k staging chunk so the first
                    # k-projection matmuls start ~5us earlier.
                    load_wT_grouped(WvT, wvt_d)
                in_proj(WkT, kTt, khT_a, khT_b, n0)
                vTt = xtp.tile([128, CC, SC], BF16, tag="xT", name="vTt")
                nc.gpsimd.dma_start(
                    vTt[:],
                    vt_d[:, n0:n0 + SC].rearrange("(cc p) n -> p cc n", p=128))
                for ntl in range(SC // 128):
                    nt = sc * (SC // 128) + ntl
                    p = big_tile("pv")
                    for cc in range(CC):
                        xw = vTt[:, cc, ntl * 128:(ntl + 1) * 128]
                        nc.tensor.matmul(p[:, 0:HF], xw, WvT[:, cc, 0:HF],
                                         start=(cc == 0), stop=(cc == CC - 1))
                        nc.tensor.matmul(p[:, HF:C], xw, WvT[:, cc, HF:C],
                                         start=(cc == 0), stop=(cc == CC - 1))
                    nc.vector.tensor_copy(
                        vh[nt][:, :, 0:DH],
                        p[:, 0:C].rearrange("p (h d) -> p h d", h=H))
                    nc.vector.tensor_copy(
                        vh[nt][:, :, DH:DH + 1],
                        ones_col[:].rearrange("p (h o) -> p h o", h=H))
                if sc == 0:
                    load_wq()
                    load_wp_bias()

        # ---- phase 2: per-superchunk attention + output projection --------
        qhp = ctx.enter_context(tc.tile_pool(name="qhp", bufs=1))
        esp = ctx.enter_context(tc.tile_pool(name="esp", bufs=3))
        xop = ctx.enter_context(tc.tile_pool(name="xop", bufs=1))
        scp = ctx.enter_context(tc.tile_pool(name="scp", bufs=2))
        yp = ctx.enter_context(tc.tile_pool(name="yp", bufs=2))
        bcp = ctx.enter_context(tc.tile_pool(name="bcp", bufs=1))

        qhT_a = [qhp.tile([128, SC], BF16, tag=f"qha{h}", name=f"qha{h}")
                 for h in range(H)]
        qhT_b = [qhp.tile([128, SC], BF16, tag=f"qhb{h}", name=f"qhb{h}")
                 for h in range(H)]
        # raw (unnormalized) U, f32 to keep a single bf16 rounding on x
        ur_a = [xop.tile([128, SC], F32, tag=f"ura{h}", name=f"ura{h}")
                for h in range(H)]
        ur_b = [xop.tile([128, SC], F32, tag=f"urb{g}", name=f"urb{g}")
                for g in range(2)]
        rs = [xop.tile([1, SC], F32, tag=f"rs{h}", name=f"rs{h}")
              for h in range(H)]
        xT_a = [xop.tile([128, SC], BF16, tag=f"xta{h}", name=f"xta{h}")
                for h in range(H)]
        xT_b = [xop.tile([128, SC], BF16, tag=f"xtb{g}", name=f"xtb{g}")
                for g in range(2)]

        def q_load(sc):
            n0 = sc * SC
            qTt = xtp.tile([128, CC, SC], BF16, tag="xT", name="qTt")
            nc.gpsimd.dma_start(
                qTt[:],
                qt_d[:, n0:n0 + SC].rearrange("(cc p) n -> p cc n", p=128))
            return qTt

        def finalize_recip(fh, src=None):
            # 1-partition RECIPROCAL of the rowsum row on DVE (iterative
            # divide, ~6.7us at 1024 wide) - slow but fully hidden under the
            # next head's score loop / the output projection.
            recip = scp.tile([1, SC], F32R, tag="recip", name="recip")
            with nc.allow_low_precision(reason="softmax denom recip f32r"):
                nc.vector.reciprocal(recip[:],
                                     rs[fh][:] if src is None else src)
            return recip

        def finalize_muls(fh, recip):
            # broadcast 1/rowsum across partitions with a rank-1 ones matmul,
            # then normalize the raw U tiles into the bf16 stationaries for
            # the output projection.
            g, blo = fh // 2, (fh % 2) * 64
            bc_ps = big_tile("bcp")
            nc.tensor.matmul(bc_ps[:, 0:HF], ones_row[:], recip[:, 0:HF],
                             start=True, stop=True)
            nc.tensor.matmul(bc_ps[:, HF:SC], ones_row[:], recip[:, HF:SC],
                             start=True, stop=True)
            bc = scp.tile([128, SC], F32, tag="bc", name="bc")
            nc.scalar.copy(bc[:], bc_ps[:])
            nc.vector.tensor_mul(xT_a[fh][:], ur_a[fh][:], bc[:])
            nc.vector.tensor_mul(xT_b[g][blo:blo + 64, :],
                                 ur_b[g][blo:blo + 64, :], bc[blo:blo + 64, :])

        def attention(sc):
            # (h, kt) flattened with the score stream leading the av stream
            # by two groups, so the next head's first exps are already in
            # flight when its avs begin - no per-head-boundary PE gap.
            def scores(h, kt):
                g, blo = h // 2, (h % 2) * 64
                s = big_tile("s")
                wa = khT_a[h][:, kt * 128:(kt + 1) * 128]
                qa = qhT_a[h]
                nc.tensor.matmul(s[:, 0:HF], wa, qa[:, 0:HF],
                                 start=True, stop=False)
                nc.tensor.matmul(s[:, HF:SC], wa, qa[:, HF:SC],
                                 start=True, stop=False)
                wb, qb = khT_b[h], qhT_b[h]
                nc.tensor.matmul(s[:, 0:HF],
                                 wb[0:64, kt * 128:(kt + 1) * 128],
                                 qb[0:64, 0:HF], start=False, stop=True)
                nc.tensor.matmul(s[:, HF:SC],
                                 wb[64:128, kt * 128:(kt + 1) * 128],
                                 qb[64:128, HF:SC], start=False, stop=True)
                es = esp.tile([128, SC], BF16, tag="es", name="es")
                nc.scalar.activation(es[:], s[:], AF.Exp, scale=SCALE)
                return es

            def av(h, kt, u_a, u_b, es):
                va = vh[kt][:, h, 0:128]
                st, sp = (kt == 0), (kt == KT - 1)
                nc.tensor.matmul(u_a[:, 0:HF], va, es[:, 0:HF],
                                 start=st, stop=sp)
                nc.tensor.matmul(u_a[:, HF:SC], va, es[:, HF:SC],
                                 start=st, stop=sp)
                vb = vh[kt][:, h, 128:DH + 1]
                nc.tensor.matmul(u_b[:, 0:HF], vb, es[:, 0:HF],
                                 start=st, stop=sp)
                nc.tensor.matmul(u_b[:, HF:SC], vb, es[:, HF:SC],
                                 start=st, stop=sp)

            seq = [(h, kt) for h in range(H) for kt in range(KT)]
            es_q = [scores(*seq[0]), scores(*seq[1])]
            pend = None
            u = None
            for j, (h, kt) in enumerate(seq):
                if j + 2 < len(seq):
                    es_q.append(scores(*seq[j + 2]))
                if kt == 0:
                    u = (ps.tile([128, SC], F32, tag="ua", name="ua", bufs=1),
                         ps.tile([65, SC], F32, tag="ub", name="ub", bufs=1))
                av(h, kt, *u, es_q.pop(0))
                if kt == 2 and pend is not None:
                    pend = (pend[0], finalize_recip(pend[0]))
                elif kt == 8 and pend is not None:
                    finalize_muls(*pend)
                    pend = None
                elif kt == KT - 1:
                    # evacuate raw U immediately so the single-buffered U
                    # psum frees before the next head's first av matmul.
                    # The rowsum row stays in psum for the LAST head: its
                    # reciprocal runs during the projection phase, when no
                    # next av needs the ub buffer, and skipping the staging
                    # copy shortens the normalization chain.
                    g, blo = h // 2, (h % 2) * 64
                    nc.scalar.copy(ur_a[h][:], u[0][:])
                    nc.vector.tensor_copy(ur_b[g][blo:blo + 64, :],
                                          u[1][0:64, :])
                    if h != H - 1:
                        nc.vector.tensor_copy(rs[h][:], u[1][64:65, :])
                    pend = (h,)
            return pend[0], u[1]

        def q_proj_groups(qTt, scalar_evac=True):
            # one projection group per output j-chunk; emitted lazily so the
            # caller can interleave them with the output projection. PSUM
            # evacuation on ScalarE when these groups run while the pending
            # head's 6.5us RECIPROCAL occupies the DVE FIFO (a DVE evac
            # there would stall the PE's psum-buffer rotation behind it);
            # on DVE for the pre-loop instance, where ScalarE copies would
            # instead delay the first attention exps.
            def group(jc):
                p = big_tile("pq")
                for cc in range(CC):
                    w = WqT[:, cc, jc * 128:(jc + 1) * 128]
                    nc.tensor.matmul(p[:, 0:HF], w, qTt[:, cc, 0:HF],
                                     start=(cc == 0), stop=(cc == CC - 1))
                    nc.tensor.matmul(p[:, HF:SC], w, qTt[:, cc, HF:SC],
                                     start=(cc == 0), stop=(cc == CC - 1))
                cp = nc.scalar.copy if scalar_evac else nc.vector.tensor_copy
                evac_jc(p, jc, qhT_a, qhT_b, 0, SC, cp, nc.scalar.copy)
            return [lambda jc=jc: group(jc) for jc in range(CC)]

        def final_part(p, ntl, heads, start, stop):
            # the two heads of a pair share xT_b[g]/WpT_b[g] partition
            # packing, and their b contributions sum into the same output -
            # one full-K=128 matmul replaces two 64-K ones when both heads
            # are present (half the streamed columns).
            ops = [("a", h) for h in heads]
            for g in range(2):
                pair = [h for h in heads if h // 2 == g]
                if len(pair) == 2:
                    ops.append(("b2", g))
                elif pair:
                    ops.append(("b1", pair[0]))
            for i, (kind, v) in enumerate(ops):
                st = start and i == 0
                sp = stop and i == len(ops) - 1
                if kind == "a":
                    w = xT_a[v][:, ntl * 128:(ntl + 1) * 128]
                    ra, rb = WpT_a[:, v, 0:HF], WpT_a[:, v, HF:C]
                elif kind == "b2":
                    w = xT_b[v][:, ntl * 128:(ntl + 1) * 128]
                    ra, rb = WpT_b[v][:, 0:HF], WpT_b[v][:, HF:C]
                else:
                    g, blo = v // 2, (v % 2) * 64
                    w = xT_b[g][blo:blo + 64, ntl * 128:(ntl + 1) * 128]
                    ra = WpT_b[g][blo:blo + 64, 0:HF]
                    rb = WpT_b[g][blo:blo + 64, HF:C]
                nc.tensor.matmul(p[:, 0:HF], w, ra, start=st, stop=sp)
                nc.tensor.matmul(p[:, HF:C], w, rb, start=st, stop=sp)

        def final_evac(p, sc, ntl):
            n0 = sc * SC
            ysb = yp.tile([128, C], F32, tag="y", name="y")
            nc.vector.tensor_add(ysb[:], p[:, 0:C], bias_bc[:])
            nc.sync.dma_start(
                y_d[n0 + ntl * 128:n0 + (ntl + 1) * 128, :], ysb[:])

        def final_ntl(sc, ntl):
            p = big_tile("py")
            final_part(p, ntl, list(range(H)), True, True)
            final_evac(p, sc, ntl)

        def finalize_tail(fh, ub_last):
            # no cover work exists after the last superchunk's attention, so
            # split the pending head's normalization into column quarters -
            # final_ntl(ntl) only needs xT columns [ntl*128,(ntl+1)*128), so
            # the output projection starts after a quarter of the reciprocal,
            # read straight from the rowsum row still sitting in psum.
            # The broadcast psum rides the now-idle "ua" bank pair: the "big"
            # bufs hold the two open part_a groups (deadlock otherwise).
            g, blo = fh // 2, (fh % 2) * 64
            recip = scp.tile([1, SC], F32R, tag="recip", name="recip")
            bc = bcp.tile([128, SC], F32, tag="bcr", name="bcr")
            bc_ps = ps.tile([128, SC], F32, tag="ua", name="bc_ps", bufs=1)
            for pi in range(4):
                lo, hi = pi * (SC // 4), (pi + 1) * (SC // 4)
                with nc.allow_low_precision(reason="softmax denom recip"):
                    nc.vector.reciprocal(recip[:, lo:hi],
                                         ub_last[64:65, lo:hi])
                nc.tensor.matmul(bc_ps[:, lo:hi], ones_row[:],
                                 recip[:, lo:hi], start=True, stop=True)
                nc.scalar.copy(bc[:, lo:hi], bc_ps[:, lo:hi])
                nc.vector.tensor_mul(xT_a[fh][:, lo:hi], ur_a[fh][:, lo:hi],
                                     bc[:, lo:hi])
                nc.vector.tensor_mul(xT_b[g][blo:blo + 64, lo:hi],
                                     ur_b[g][blo:blo + 64, lo:hi],
                                     bc[blo:blo + 64, lo:hi])

        # q-superchunk pipeline: attention(sc) leaves the last head's
        # normalization pending; the next superchunk's q-projection groups
        # are PE work independent of it and cover the reciprocal chain, then
        # the output projection consumes the normalized stationaries.
        qTt = q_load(0)
        for fn in q_proj_groups(qTt, scalar_evac=False):
            fn()
        for sc in range(NSC):
            if sc + 1 < NSC:
                qt_next = q_load(sc + 1)
            fh, ub_last = attention(sc)
            if sc + 1 < NSC:
                recip = finalize_recip(fh, src=ub_last[64:65, :])
                qp = q_proj_groups(qt_next)
                for fn in qp[0:4]:
                    fn()
                finalize_muls(fh, recip)
                rest = qp[4:]
                for ntl in range(SC // 128):
                    final_ntl(sc, ntl)
                    if rest:
                        rest.pop(0)()
            else:
                # tail: open the first two output-projection groups with the
                # already-normalized heads as cover work, weave the pending
                # head's quartered normalization in, then close them.
                heads_a = [h for h in range(H) if h != fh]
                p0 = big_tile("py")
                final_part(p0, 0, heads_a, True, False)
                p1 = big_tile("py")
                final_part(p1, 1, heads_a, True, False)
                finalize_tail(fh, ub_last)
                final_part(p0, 0, [fh], False, True)
                final_evac(p0, sc, 0)
                final_part(p1, 1, [fh], False, True)
                final_evac(p1, sc, 1)
                for ntl in range(2, SC // 128):
                    final_ntl(sc, ntl)

    nc.compile()
    return nc


def _get_built():
    global _BUILT
    if _BUILT is None:
        _BUILT = _build()
    return _BUILT


def run(inputs, trace=False, **kw):
    """Run on all 8 cores; returns (y [B,N,C] float32, BassKernelResults)."""
    import ml_dtypes
    from concourse.bass_utils import run_bass_kernel_spmd

    nc = _get_built()
    bf16 = ml_dtypes.bfloat16
    f32 = np.float32
    wpt = np.asarray(inputs["Wp"], f32).T  # [c', j]
    wpt_packed = np.concatenate(
        [wpt[h * DH:h * DH + 128] for h in range(H)]
        + [wpt[h * DH + 128:(h + 1) * DH] for h in range(H)])
    shared = {
        "WqT": np.ascontiguousarray(np.asarray(inputs["Wq"], f32).T).astype(bf16),
        "WkT": np.ascontiguousarray(np.asarray(inputs["Wk"], f32).T).astype(bf16),
        "WvT": np.ascontiguousarray(np.asarray(inputs["Wv"], f32).T).astype(bf16),
        "WpT": np.ascontiguousarray(wpt_packed).astype(bf16),
        "bp": np.ascontiguousarray(np.asarray(inputs["bp"], f32)),
    }
    q = np.asarray(inputs["q"], f32)
    k = np.asarray(inputs["k"], f32)
    v = np.asarray(inputs["v"], f32)
    in_maps = []
    for b in range(B):
        m = dict(shared)
        m["qT"] = np.ascontiguousarray(q[b].T).astype(bf16)
        m["kT"] = np.ascontiguousarray(k[b].T).astype(bf16)
        m["vT"] = np.ascontiguousarray(v[b].T).astype(bf16)
        in_maps.append(m)
    res = run_bass_kernel_spmd(nc, in_maps, list(range(B)), trace=trace, **kw)
    y = np.stack([res.results[b]["y"] for b in range(B)]).astype(np.float32)
    return y, res


def kernel(q, k, v, Wq, Wk, Wv, Wp, bp):
    y, _ = run({"q": q, "k": k, "v": v, "Wq": Wq, "Wk": Wk, "Wv": Wv,
                "Wp": Wp, "bp": bp})
    return y
